# revision 1
# baseline (speedup 1.0000x reference)
"""Distributed MHA kernel for Trainium2 (8 NeuronCores).

Problem: x,f:(2,2048,1024), W_qkv:(1024,3072), W_proj:(1024,1024), H=16 heads.
reference returns (out, attn2gcn) with
  attn2gcn = softmax(q k^T / sqrt(64)) v   (per head, concat over heads)
  out      = (attn2gcn + f) @ W_proj + b_proj

Sharding: tensor-parallel over heads — core c owns heads 2c, 2c+1 for both
batches (column block c*128 of the hidden dim).  Attention arithmetic is
bf16 matmuls with fp32 PSUM (the attn output's max-abs rel-err budget is
too tight for fp8 anywhere on that path); softmax stays fp32 on ACT.

The projection is split: out = f @ W_proj + attn2gcn @ W_proj + b.
  - f @ W_proj runs in bf16 against host-staged fT/W tiles.  It has no
    on-device dependencies at all, so its 8 m-passes are the PE filler
    that keeps the tensor engine hot through the collective windows
    (the HAM governor halves the PE clock after any idle window).
  - attn2gcn (avn) rides the AllToAll as bf16 (x32; fp8 collectives
    measured pathologically slow), is cast to fp8e4 after the reshard,
    and the contraction runs as fp8 DoubleRow matmuls: per head-parity
    half just 2 instructions per m-tile (contraction 2x128 per instr at
    0.5 cycles/row), so the work after the last AllToAll is ~4k PE
    cycles instead of the old 33k.
  - scales: f, W_proj staged x32 (bf16), avn x32 (fp8) -> psum carries
    1024*out; the host divides by 1024 after gathering (pure numpy).

Per-core dataflow (as in the tuned baseline): qkvT = W-slice^T @ x^T in
head-packed tiles; batch-1 qkv emitted inside head 0's early attention
chunks as PE filler; attention per (head, batch) in scoresT layout,
software-pipelined so the PE runs ahead of the ACT exp; av^T accumulates
an all-ones 65th v column giving the softmax denominator for free;
normalization for chunk c-1 is emitted inside chunk c off the PE
critical path.  The head-0 AllToAll half fires mid-kernel; its DoubleRow
contraction and the f@W fillers interleave into later chunks; the head-1
half completes at the end with only the tiny DoubleRow tail behind it.
"""

import numpy as np

B, N, C, H, D = 2, 2048, 1024, 16, 64
BN = B * N
SCALE = D ** -0.5
N_CORES = 8
KT = C // 128      # 8 contraction tiles
NCH = BN // 512    # 8 qkv free chunks
PS = 32.0          # fp8/bf16 staging scale for the projection operands

_cached = None


def _build():
    from contextlib import ExitStack

    import concourse.mybir as mybir
    import concourse.tile as tile
    from concourse import bacc
    from concourse.masks import make_identity

    F32 = mybir.dt.float32
    BF16 = mybir.dt.bfloat16
    F8 = mybir.dt.float8e4
    EXP = mybir.ActivationFunctionType.Exp
    COPY = mybir.ActivationFunctionType.Copy
    DR = mybir.MatmulPerfMode.DoubleRow

    nc = bacc.Bacc("TRN2", target_bir_lowering=False, debug=False,
                   num_devices=N_CORES)

    xT_ext = nc.dram_tensor("xT", [C, BN], BF16, kind="ExternalInput").ap()
    wkqv_ext = nc.dram_tensor("wkqv", [C, 384], BF16, kind="ExternalInput").ap()
    fT_ext = nc.dram_tensor("fT", [C, 512], BF16, kind="ExternalInput").ap()
    wproj_ext = nc.dram_tensor("wproj", [C, C], BF16, kind="ExternalInput").ap()
    wp8_ext = [nc.dram_tensor(f"wp8_{h}", [128, 4 * C], mybir.dt.float8e4,
                              kind="ExternalInput").ap() for h in range(2)]
    bprojT_ext = nc.dram_tensor("bprojT", [128, 8], F32, kind="ExternalInput").ap()
    attn_t_ext = nc.dram_tensor("attn_t", [128, BN], BF16, kind="ExternalOutput").ap()
    out_t_ext = nc.dram_tensor("out_t", [C, 512], BF16, kind="ExternalOutput").ap()

    groups = [list(range(N_CORES))]

    with tile.TileContext(nc) as tc:
        with ExitStack() as octx:
            pp = octx.enter_context(tc.tile_pool(name="persist", bufs=1))
            kqp = octx.enter_context(tc.tile_pool(name="kq", bufs=1))
            vap = octx.enter_context(tc.tile_pool(name="vaug", bufs=1))
            vtp = octx.enter_context(tc.tile_pool(name="vt", bufs=1))
            wqp = octx.enter_context(tc.tile_pool(name="wq", bufs=1))
            xsp = octx.enter_context(tc.tile_pool(name="xs", bufs=16))
            dram = octx.enter_context(
                tc.tile_pool(name="dram", bufs=1, space="DRAM"))
            # qkv weights and the first x chunk interleaved at the head of
            # the DMA queues: matmul k of phase Q needs only (wq[k], xs0[k]),
            # so the first matmul can start after ~2 transfers instead of
            # waiting out the full weight set's issue latency
            wq_sb = []
            xs0_t = []
            for k in range(KT):
                w = wqp.tile([128, 384], BF16, name=f"wq{k}")
                nc.sync.dma_start(w[:], wkqv_ext[k * 128:(k + 1) * 128, :])
                wq_sb.append(w)
                xs = xsp.tile([128, 512], BF16, name="xs", tag="xs")
                nc.sync.dma_start(xs[:], xT_ext[k * 128:(k + 1) * 128, 0:512])
                xs0_t.append(xs)

            ident = pp.tile([128, 128], BF16)
            make_identity(nc, ident[:])
            # 32, not 1: the normalization broadcast then yields avn*32
            # directly (the x32 the a2a/proj stage wants); the host divides
            # the attn output by 32 after gathering.
            ones64b = pp.tile([1, 64], BF16)
            nc.vector.memset(ones64b[:], PS)

            kT = kqp.tile([128, BN], BF16, name="kT")
            qT = kqp.tile([128, BN], BF16, name="qT")
            vT = vtp.tile([128, BN], BF16, name="vT")
            mtiles = [kT, qT, vT]
            v_aug = [[vap.tile([128, 65], BF16, name=f"va{h}_{j}")
                      for j in range(32)] for h in range(2)]

            # avn is quantized to fp8 before the reshard; the collective
            # itself runs on a bf16 VIEW of those bytes ([512, 256] bf16 ==
            # [512, 512] fp8) — fp8-dtype collectives measured ~20x slower,
            # and this also removes any post-a2a convert from the tail.
            a2a_in = [dram.tile([512, 256], BF16, name=f"a2ain{hh}")
                      for hh in range(2)]
            a2a_out = [dram.tile([512, 256], BF16, name=f"a2aout{hh}")
                       for hh in range(2)]

            # tiny warmup collective: the first AllToAll of a NEFF pays a
            # ~11us CC-stream start delay; paying it here (overlapped with
            # phase Q / the runtime barrier) takes it off the mid-kernel
            # critical path
            warm_in = dram.tile([8, 256], BF16, name="warm_in")
            warm_out = dram.tile([8, 256], BF16, name="warm_out")
            warm_sb = pp.tile([8, 256], BF16, name="warm_sb")
            nc.vector.memset(warm_sb[:], 0.0)
            nc.sync.dma_start(warm_in[:], warm_sb[:])
            nc.gpsimd.collective_compute(
                "AllToAll", mybir.AluOpType.bypass,
                replica_groups=groups,
                ins=[warm_in.opt()], outs=[warm_out.opt()])

            def qkv_transposes(nch, tpool, ttag):
                for j in range(4 * nch, 4 * nch + 4):
                    tps = tpool.tile([128, 128], BF16, name="tps", tag=ttag)
                    nc.tensor.transpose(
                        tps[:], vT[:, j * 128:(j + 1) * 128], ident[:])
                    for h in range(2):
                        nc.vector.tensor_copy(
                            v_aug[h][j][:, 0:64], tps[:, h * 64:(h + 1) * 64])
                        nc.vector.memset(v_aug[h][j][:, 64:65], 1.0)

            def qkv_xs(nch):
                xs_t = []
                for k in range(KT):
                    xs = xsp.tile([128, 512], BF16, name="xs", tag="xs")
                    nc.sync.dma_start(
                        xs[:], xT_ext[k * 128:(k + 1) * 128,
                                      nch * 512:(nch + 1) * 512])
                    xs_t.append(xs)
                return xs_t

            # ------------- phase Q: qkv for the first two chunks -------------
            with ExitStack() as qctx:
                qps = qctx.enter_context(
                    tc.tile_pool(name="qkv_ps", bufs=1, space="PSUM"))
                trp = qctx.enter_context(
                    tc.tile_pool(name="tr_ps", bufs=2, space="PSUM"))
                for nch in range(2):
                    xs_t = xs0_t if nch == 0 else qkv_xs(nch)
                    psums = [qps.tile([128, 512], F32, name=f"qps{m}",
                                      tag=f"qps{m}") for m in range(3)]
                    for k in range(KT):
                        for m in range(3):
                            nc.tensor.matmul(
                                psums[m][:],
                                wq_sb[k][:, m * 128:(m + 1) * 128],
                                xs_t[k][:], start=(k == 0), stop=(k == KT - 1))
                    for m in range(3):
                        nc.vector.tensor_copy(
                            mtiles[m][:, nch * 512:(nch + 1) * 512],
                            psums[m][:])
                    if nch == 0:
                        # chunk 1's transposes are deferred into attention
                        # chunk 0 (they are only consumed from kj==4 on) so
                        # the PE is not parked on this DVE chain at the
                        # phase transition
                        qkv_transposes(nch, trp, "tps")

            # loads needed by the f@W fillers / normalization, queued behind
            # the phase-Q traffic but ahead of the deferred-qkv x chunks
            bias_sb = pp.tile([128, 8], F32)
            nc.sync.dma_start(bias_sb[:], bprojT_ext[:])
            fT_sb = []
            for t in range(KT):
                ft = pp.tile([128, 512], BF16, name=f"fTsb{t}")
                nc.sync.dma_start(ft[:], fT_ext[t * 128:(t + 1) * 128, :])
                fT_sb.append(ft)

            # ---------------- phase A: attention + fillers ----------------
            with ExitStack() as actx:
                expp = actx.enter_context(tc.tile_pool(name="exp", bufs=4))
                avup = actx.enter_context(tc.tile_pool(name="avu", bufs=3))
                normp = actx.enter_context(tc.tile_pool(name="norm", bufs=2))
                avnp = actx.enter_context(tc.tile_pool(name="avn", bufs=2))
                oaccp = actx.enter_context(tc.tile_pool(name="oacc", bufs=1))
                wpp = actx.enter_context(tc.tile_pool(name="wp", bufs=1))
                wp8p = actx.enter_context(tc.tile_pool(name="wp8", bufs=1))
                rhs8p = actx.enter_context(tc.tile_pool(name="rhs8", bufs=1))
                sps = actx.enter_context(
                    tc.tile_pool(name="scores_ps", bufs=2, space="PSUM"))
                avps = actx.enter_context(
                    tc.tile_pool(name="av_ps", bufs=1, space="PSUM"))
                bcps = actx.enter_context(
                    tc.tile_pool(name="bc_ps", bufs=1, space="PSUM"))
                pjps = actx.enter_context(
                    tc.tile_pool(name="pj_ps", bufs=1, space="PSUM"))
                out_acc = [oaccp.tile([128, 512], F32, name=f"oacc{m}")
                           for m in range(8)]
                # prefetch projection weights during early attention
                wp_sb = []
                for t in range(KT):
                    w = wpp.tile([128, C], BF16, name=f"wp_{t}")
                    nc.sync.dma_start(w[:], wproj_ext[t * 128:(t + 1) * 128, :])
                    wp_sb.append(w)
                wp8_sb = []
                for hh in range(2):
                    w8 = wp8p.tile([128, 4, C], F8, name=f"wp8_{hh}")
                    nc.sync.dma_start(w8[:], wp8_ext[hh][:])
                    wp8_sb.append(w8)

                qkv_work = [(nch, m) for nch in range(2, 8) for m in range(3)]
                pe_work = []   # 4-m-tile DoubleRow units for the h0 half
                rhs8_sb = {}

                qkv_xs_cache = {}

                def qkv_deferred_unit(unit):
                    """One m-pass of a deferred qkv chunk — small PE
                    filler emitted inside head 0's early attention chunks
                    (single pj psum slot)."""
                    nch, m = unit
                    if nch not in qkv_xs_cache:
                        qkv_xs_cache[nch] = qkv_xs(nch)
                    xs_t = qkv_xs_cache[nch]
                    pjt = pjps.tile([128, 512], F32, name="qkvd", tag="pj")
                    for k in range(KT):
                        nc.tensor.matmul(
                            pjt[:], wq_sb[k][:, m * 128:(m + 1) * 128],
                            xs_t[k][:], start=(k == 0), stop=(k == KT - 1))
                    nc.vector.tensor_copy(
                        mtiles[m][:, nch * 512:(nch + 1) * 512], pjt[:])
                    if m == 2:
                        qkv_transposes(nch, bcps, "bc")
                        del qkv_xs_cache[nch]

                fw_state = {"m": 0, "k": 0, "pj": None}

                def fw_step(nmm):
                    """Emit up to nmm f @ W_proj matmuls (dependency-free
                    bf16 PE filler, fine-grained).  Injected exactly where
                    the PE would otherwise wait on the ACT exp — chunk
                    boundaries — so the filler cost hides in the bubble.
                    4 matmuls at kj==15 + 4 at the next chunk's kj==0
                    complete one m-pass without overlapping the other
                    users of the shared pj psum slot."""
                    st = fw_state
                    while nmm > 0 and st["m"] < 8:
                        m, k = st["m"], st["k"]
                        if k == 0:
                            st["pj"] = pjps.tile([128, 512], F32,
                                                 name="pjf", tag="pj")
                        nc.tensor.matmul(
                            st["pj"][:], wp_sb[k][:, m * 128:(m + 1) * 128],
                            fT_sb[k][:], start=(k == 0), stop=(k == KT - 1))
                        nmm -= 1
                        st["k"] += 1
                        if st["k"] == KT:
                            nc.vector.tensor_scalar_add(
                                out_acc[m][:], st["pj"][:],
                                bias_sb[:, m:m + 1])
                            st["k"] = 0
                            st["m"] += 1

                def load_rhs8(hh):
                    """Stack the 8 received [64,512] bf16 tiles of half hh
                    into two DoubleRow rhs tiles [128, 2, 512] and cast to
                    fp8 on DVE (values are already x32-scaled).

                    Emit only once the collective is certainly complete:
                    these DMAs carry a wait on the a2a output and would
                    otherwise stall the sync DMA queue for everything
                    emitted after them."""
                    tiles = []
                    for u in range(2):
                        r = rhs8p.tile([128, 2, 512], F8, name=f"r8_{hh}{u}")
                        for i in range(2):
                            s = 2 * u + i
                            # blocks 2s and 2s+1 are adjacent rows: one DMA
                            # fills both partition halves of slot i (bytes
                            # are already fp8; the bf16 view just matches
                            # the collective buffer's dtype).  Issue from
                            # two HWDGE queues so the 4 loads overlap.
                            eng = nc.sync if i == 0 else nc.scalar
                            eng.dma_start(
                                r[:, i, :].bitcast(BF16),
                                a2a_out[hh][s * 128:(s + 1) * 128, :])
                        tiles.append(r)
                    rhs8_sb[hh] = tiles

                def proj8_unit(hh, ms, final):
                    """DoubleRow avn@W for head-parity half hh, m-tiles ms."""
                    if hh not in rhs8_sb:
                        load_rhs8(hh)
                    r8 = rhs8_sb[hh]
                    w8 = wp8_sb[hh]
                    for m in ms:
                        pj = pjps.tile([128, 512], F32, name="pj8", tag="pj")
                        for u in range(2):
                            nc.tensor.matmul(
                                pj[:], w8[:, 2 * u:2 * u + 2,
                                          m * 128:(m + 1) * 128],
                                r8[u][:], start=(u == 0), stop=(u == 1),
                                perf_mode=DR)
                        if final:
                            ot = avnp.tile([128, 512], BF16, name="ot",
                                           tag="ot")
                            nc.vector.tensor_tensor(
                                ot[:], pj[:], out_acc[m][:],
                                mybir.AluOpType.add)
                            nc.sync.dma_start(
                                out_t_ext[m * 128:(m + 1) * 128, :], ot[:])
                        else:
                            nc.vector.tensor_tensor(
                                out_acc[m][:], pj[:], out_acc[m][:],
                                mybir.AluOpType.add)

                def norm_pre(avu):
                    """1/denom chain — latency starts at chunk end.  The
                    reciprocal runs in a [128, 8] spread of the denominator
                    row (DVE cost is free-size only: 8 cycles, not 1024);
                    both DMAs use the same p-major element order."""
                    dn = normp.tile([128, 8], F32, name="dn", tag="dn")
                    nc.sync.dma_start(dn[:], avu[64:65, :])
                    dninv = normp.tile([128, 8], F32, name="dninv",
                                       tag="dninv")
                    nc.vector.reciprocal_approx_fast(dninv[:], dn[:])
                    dnb = normp.tile([128, 8], BF16, name="dnb", tag="dnb")
                    # on DVE, not ACT: an ACT copy would make every next
                    # chunk's exps queue behind this chain (ACT is in-order)
                    nc.vector.tensor_copy(dnb[:], dninv[:])
                    dinvb = normp.tile([1, 1024], BF16, name="dinvb",
                                       tag="dinvb")
                    nc.sync.dma_start(dinvb[:], dnb[:])
                    return dinvb

                def norm_chunk(h, b, ch, avu, dinvb):
                    """avn32 = 32*avu[0:64]/avu[64]; attn_t out (x32, the
                    host divides); bf16 a2a staging (x32 by design)."""
                    po = h * 64
                    cs = b * 2048 + ch * 1024
                    avn32 = avnp.tile([64, 1024], BF16, name="avn32",
                                      tag="avn32")
                    avn8 = avnp.tile([64, 1024], F8, name="avn8", tag="avn8")
                    for s in range(2):
                        bc = bcps.tile([64, 512], F32, name="bc", tag="bc")
                        nc.tensor.matmul(bc[:], ones64b[:],
                                         dinvb[:, s * 512:(s + 1) * 512],
                                         start=True, stop=True)
                        # fp8 staging product first: the collective waits
                        # on it, the bf16 attn copy can trail
                        nc.vector.tensor_tensor(
                            avn8[:, s * 512:(s + 1) * 512],
                            avu[0:64, s * 512:(s + 1) * 512],
                            bc[:], mybir.AluOpType.mult)
                        nc.vector.tensor_tensor(
                            avn32[:, s * 512:(s + 1) * 512],
                            avu[0:64, s * 512:(s + 1) * 512],
                            bc[:], mybir.AluOpType.mult)
                    for j in (cs // 512, cs // 512 + 1):
                        off = j * 512 - cs
                        nc.sync.dma_start(
                            a2a_in[h][j * 64:(j + 1) * 64, :],
                            avn8[:, off:off + 512].bitcast(BF16))
                    nc.sync.dma_start(
                        attn_t_ext[po:po + 64, cs:cs + 1024], avn32[:])

                def mm1_kj(h, b, cs, kj, scores_q):
                    po = h * 64
                    jt = b * 16 + kj
                    sc = sps.tile([128, 1024], F32, name="scores", tag="sc")
                    for s in range(2):
                        nc.tensor.matmul(
                            sc[:, s * 512:(s + 1) * 512],
                            kT[po:po + 64, jt * 128:(jt + 1) * 128],
                            qT[po:po + 64, cs + s * 512:cs + (s + 1) * 512],
                            start=True, stop=True)
                    scores_q[kj] = sc

                def fire_a2a(h):
                    nc.gpsimd.collective_compute(
                        "AllToAll", mybir.AluOpType.bypass,
                        replica_groups=groups,
                        ins=[a2a_in[h].opt()], outs=[a2a_out[h].opt()])
                    if h == 0:
                        pe_work.extend([(0, range(0, 4)), (0, range(4, 8))])

                def do_chunk(ci, h, b, ch, pending):
                    cs = b * 2048 + ch * 1024
                    av = avps.tile([128, 1024], F32, name="av", tag="av")
                    scores_q = {}
                    mm1_kj(h, b, cs, 0, scores_q)
                    for kj in range(16):
                        if kj + 1 < 16:
                            mm1_kj(h, b, cs, kj + 1, scores_q)
                        if kj == 4 and pending is not None:
                            ph = pending[0]
                            norm_chunk(*pending)
                            pending = None
                            if ph != h:
                                # that was the previous head's last chunk:
                                # its AllToAll half can fire now
                                fire_a2a(ph)
                        if ci == 0 and kj == 1:
                            qkv_transposes(1, bcps, "bc")
                        if ci < 3 and kj in (1, 3, 5) and qkv_work:
                            qkv_deferred_unit(qkv_work.pop(0))
                            if qkv_work:
                                qkv_deferred_unit(qkv_work.pop(0))
                        # f@W boundary filler: runs while the PE waits on
                        # this chunk's last exp / the next chunk's first one;
                        # the last boundary's share stays in reserve as
                        # post-loop PE cover
                        if kj == 15 or (kj == 0 and 1 <= ci < 7):
                            fw_step(4)
                        sc = scores_q.pop(kj)
                        ex = expp.tile([128, 1024], BF16, name="ex", tag="ex")
                        nc.scalar.activation(ex[:], sc[:], EXP, scale=SCALE)
                        jt = b * 16 + kj
                        for s in range(2):
                            nc.tensor.matmul(
                                av[0:65, s * 512:(s + 1) * 512],
                                v_aug[h][jt][:],
                                ex[:, s * 512:(s + 1) * 512],
                                start=(kj == 0), stop=(kj == 15))
                    avu = avup.tile([65, 1024], F32, name="avu", tag="avu")
                    nc.vector.tensor_copy(avu[:], av[0:65, :])
                    return (h, b, ch, avu, norm_pre(avu))

                pending = None
                ci = 0
                for h in range(2):
                    for b in range(2):
                        for ch in range(2):
                            pending = do_chunk(ci, h, b, ch, pending)
                            ci += 1
                # remaining f@W work bridges the PE from the last av matmul
                # over the final norm chain; then the last chunk's norm,
                # its collective, and the h0 DoubleRow units as cover while
                # the a2a(1) transfer lands.  a2a-gated DMAs come last in
                # the sync queue (they park it on the collective semaphore).
                fw_step(64)
                norm_chunk(*pending)
                fire_a2a(1)
                while pe_work:
                    proj8_unit(*pe_work.pop(0), final=False)

                # always-ready burn matmuls: the list scheduler holds them
                # until nothing else is runnable — the final norm chain and
                # the a2a(1) transfer window — so PE activity there keeps
                # the HAM governor from halving the clock before the tail
                # contraction.  They live in the pj ring: the bc ring would
                # chain them BEHIND the final norm's broadcast tiles.
                for _ in range(32):
                    bp = pjps.tile([64, 512], F32, name="burn", tag="pj")
                    nc.tensor.matmul(bp[:], ones64b[:], fT_sb[0][0:1, :],
                                     start=True, stop=True)

                # tail: head-parity-1 DoubleRow contraction + output
                proj8_unit(1, range(8), final=True)

    nc.compile()
    return nc


def kernel(x, f, W_qkv, W_proj, b_proj):
    import ml_dtypes
    from concourse.bass_utils import run_bass_kernel_spmd

    global _cached
    if _cached is None:
        _cached = _build()
    nc = _cached

    BF = ml_dtypes.bfloat16
    F8 = ml_dtypes.float8_e4m3
    x = np.ascontiguousarray(np.asarray(x, dtype=np.float32))
    f = np.ascontiguousarray(np.asarray(f, dtype=np.float32))
    W_qkv = np.asarray(W_qkv, dtype=np.float32)
    W_proj = np.asarray(W_proj, dtype=np.float32)
    b_proj = np.asarray(b_proj, dtype=np.float32)

    Wq, Wk, Wv = W_qkv[:, 0:C], W_qkv[:, C:2 * C], W_qkv[:, 2 * C:3 * C]
    xT = np.ascontiguousarray(x.reshape(BN, C).T.astype(BF))
    fT = (f.reshape(BN, C).T * PS).astype(BF)
    wproj_b = np.ascontiguousarray((W_proj * PS).astype(BF))
    bprojT = np.ascontiguousarray((b_proj * PS * PS).reshape(8, 128).T)

    # DoubleRow fp8 W_proj halves: wp8_h[p, s, m] = (W_proj*PS)[row, m] with
    # row = head*64 + p%64, head = 4s + h + 2*(p//64)
    Wp32 = W_proj * PS
    p = np.arange(128)
    wp8 = []
    for h in range(2):
        rows = np.empty((128, 4), np.int64)
        for s in range(4):
            head = 4 * s + h + 2 * (p // 64)
            rows[:, s] = head * 64 + (p % 64)
        wp8.append(np.ascontiguousarray(
            Wp32[rows, :].astype(F8).reshape(128, 4 * C)))

    in_maps = []
    for c in range(N_CORES):
        cols = slice(c * 128, (c + 1) * 128)     # heads 2c, 2c+1
        wkqv = np.ascontiguousarray(np.concatenate(
            [Wk[:, cols], Wq[:, cols], Wv[:, cols]], axis=1).astype(BF))
        in_maps.append({
            "xT": xT,
            "wkqv": wkqv,
            "fT": np.ascontiguousarray(fT[:, c * 512:(c + 1) * 512]),
            "wproj": wproj_b,
            "wp8_0": wp8[0],
            "wp8_1": wp8[1],
            "bprojT": bprojT,
        })

    res = run_bass_kernel_spmd(nc, in_maps, core_ids=list(range(N_CORES)))

    attn = np.empty((BN, C), dtype=np.float32)
    out = np.empty((BN, C), dtype=np.float32)
    for c in range(N_CORES):
        r = res.results[c]
        attn[:, c * 128:(c + 1) * 128] = r["attn_t"].T.astype(np.float32) / PS
        out[c * 512:(c + 1) * 512, :] = \
            r["out_t"].T.astype(np.float32) / (PS * PS)
    return out.reshape(B, N, C), attn.reshape(B, N, C)



# revision 13
# speedup vs baseline: 1.0262x; 1.0262x over previous
"""Distributed MHA kernel for Trainium2 (8 NeuronCores).

Problem: x,f:(2,2048,1024), W_qkv:(1024,3072), W_proj:(1024,1024), H=16 heads.
reference returns (out, attn2gcn) with
  attn2gcn = softmax(q k^T / sqrt(64)) v   (per head, concat over heads)
  out      = (attn2gcn + f) @ W_proj + b_proj

Sharding: tensor-parallel over heads — core c owns heads 2c, 2c+1 for both
batches (column block c*128 of the hidden dim).  Attention arithmetic is
bf16 matmuls with fp32 PSUM (the attn output's max-abs rel-err budget is
too tight for fp8 anywhere on that path: ex/v at e4m3 alone would cost
~1.7e-2 of the 2e-2 budget); softmax stays fp32 on ACT.

Timeline model (v2): the attention phase is a balanced PE/ACT race — ACT
exp costs (1024+352)/1.2GHz = 1113ns per kj and the core PE work (2 scores
+ 2 av matmuls, F=512) is ~1050ns/kj, so every extra PE cycle in the
window extends the kernel ~1:1.  Design consequences:
  - chunk order is batch-outer (b0: h0c0,h0c1,h1c0,h1c1; then b1) so the
    deferred qkv for batch 1 has 4 chunks of slack; only chunks 0-1 run
    eagerly in phase Q.  Deferred qkv is injected ONE matmul at a time
    (not 8-matmul bursts) between the ACT-feeding mm1 and the av matmuls.
  - h0's AllToAll fires after its last chunk (ci=5, during ci=6); h1's
    fires post-loop, its transfer covered by the f@W matmuls (real work
    that replaces the old burn matmuls and keeps the HAM clock warm).
  - out_acc[m] is bias-initialized at t0 so the f@W and h0-DoubleRow
    accumulations commute (plain DVE adds, any order).
  - DMA issue costs ~590ns ON THE ISSUING ENGINE'S QUEUE: the scalar
    (ACT) HWDGE queue carries only pre-attention traffic (weights, tail
    prefetch — its issue cost drains during phase Q) plus post-attention
    stores; xs/staging/attn_t/norm ride sync; the a2a-gated rhs8 loads
    ride gpsimd SWDGE (on a HWDGE queue the list scheduler hoists them
    and parks the queue on the collective semaphore).
  - the ACT exp table load (~1.3us) is prepaid with a dummy exp at t0.

The projection is split: out = f @ W_proj + attn2gcn @ W_proj + b.
  - attn2gcn (avn) rides the AllToAll as bf16 (x32; fp8 collectives
    measured pathologically slow), is cast to fp8e4 after the reshard,
    and the contraction runs as fp8 DoubleRow matmuls.
  - scales: f, W_proj staged x32 (bf16), avn x32 (fp8) -> psum carries
    1024*out; the host divides by 1024 after gathering (pure numpy).
"""

import numpy as np

B, N, C, H, D = 2, 2048, 1024, 16, 64
BN = B * N
SCALE = D ** -0.5
N_CORES = 8
KT = C // 128      # 8 contraction tiles
NCH = BN // 512    # 8 qkv free chunks
PS = 32.0          # fp8/bf16 staging scale for the projection operands

_cached = None


def _build():
    from contextlib import ExitStack

    import concourse.mybir as mybir
    import concourse.tile as tile
    from concourse import bacc
    from concourse.masks import make_identity

    F32 = mybir.dt.float32
    BF16 = mybir.dt.bfloat16
    F8 = mybir.dt.float8e4
    EXP = mybir.ActivationFunctionType.Exp
    DR = mybir.MatmulPerfMode.DoubleRow
    ADD = mybir.AluOpType.add
    MULT = mybir.AluOpType.mult

    nc = bacc.Bacc("TRN2", target_bir_lowering=False, debug=False,
                   num_devices=N_CORES)

    xT_ext = nc.dram_tensor("xT", [C, BN], BF16, kind="ExternalInput").ap()
    wkqv_ext = nc.dram_tensor("wkqv", [C, 384], BF16, kind="ExternalInput").ap()
    fT_ext = nc.dram_tensor("fT", [C, 512], BF16, kind="ExternalInput").ap()
    wproj_ext = nc.dram_tensor("wproj", [C, C], BF16, kind="ExternalInput").ap()
    wp8_ext = [nc.dram_tensor(f"wp8_{h}", [128, 4 * C], mybir.dt.float8e4,
                              kind="ExternalInput").ap() for h in range(2)]
    bprojT_ext = nc.dram_tensor("bprojT", [128, 8], F32, kind="ExternalInput").ap()
    attn_t_ext = nc.dram_tensor("attn_t", [128, BN], BF16, kind="ExternalOutput").ap()
    out_t_ext = nc.dram_tensor("out_t", [C, 512], BF16, kind="ExternalOutput").ap()

    groups = [list(range(N_CORES))]

    with tile.TileContext(nc) as tc:
        with ExitStack() as octx:
            pp = octx.enter_context(tc.tile_pool(name="persist", bufs=1))
            kqp = octx.enter_context(tc.tile_pool(name="kq", bufs=1))
            vap = octx.enter_context(tc.tile_pool(name="vaug", bufs=1))
            vtp = octx.enter_context(tc.tile_pool(name="vt", bufs=1))
            wqp = octx.enter_context(tc.tile_pool(name="wq", bufs=1))
            xsp = octx.enter_context(tc.tile_pool(name="xs", bufs=16))
            oaccp = octx.enter_context(tc.tile_pool(name="oacc", bufs=1))
            dram = octx.enter_context(
                tc.tile_pool(name="dram", bufs=1, space="DRAM"))
            # startup DMA: qkv weights ride the scalar HWDGE queue (idle
            # until the first exp), x chunks ride sync — the two streams
            # overlap and the first matmul can start after ~1 transfer
            wq_sb = []
            xs0_t = []
            for k in range(KT):
                w = wqp.tile([128, 384], BF16, name=f"wq{k}")
                nc.scalar.dma_start(w[:], wkqv_ext[k * 128:(k + 1) * 128, :])
                wq_sb.append(w)
                xs = xsp.tile([128, 512], BF16, name="xs", tag="xs")
                nc.sync.dma_start(xs[:], xT_ext[k * 128:(k + 1) * 128, 0:512])
                xs0_t.append(xs)

            # small persistent scratch + warmup
            ident = pp.tile([128, 128], BF16)
            make_identity(nc, ident[:])
            # 32, not 1: the normalization broadcast then yields avn*32
            # directly (the x32 the a2a/proj stage wants); the host divides
            # the attn output by 32 after gathering.
            ones64b = pp.tile([1, 64], BF16)
            nc.vector.memset(ones64b[:], PS)

            # prepay the ACT exp-table load (~1.3us) before the first real
            # exp; the scalar queue is idle during phase Q anyway
            dummy = pp.tile([8, 8], BF16, name="dummy")
            nc.vector.memset(dummy[:], 0.0)
            dummy2 = pp.tile([8, 8], BF16, name="dummy2")
            nc.scalar.activation(dummy2[:], dummy[:], EXP)

            # bias for the projection accumulators (tiny, scalar queue)
            bias_sb = pp.tile([128, 8], F32)
            nc.scalar.dma_start(bias_sb[:], bprojT_ext[:])

            kT = kqp.tile([128, BN], BF16, name="kT")
            qT = kqp.tile([128, BN], BF16, name="qT")
            vT = vtp.tile([128, BN], BF16, name="vT")
            mtiles = [kT, qT, vT]
            v_aug = [[vap.tile([128, 65], BF16, name=f"va{h}_{j}")
                      for j in range(32)] for h in range(2)]

            # out_acc[m] starts at b_proj*PS*PS; f@W and the h0 DoubleRow
            # passes then += into it in any order (DVE adds commute)
            out_acc = [oaccp.tile([128, 512], F32, name=f"oacc{m}")
                       for m in range(8)]
            for m in range(8):
                nc.vector.memset(out_acc[m][:], 0.0)
                nc.vector.tensor_scalar_add(
                    out_acc[m][:], out_acc[m][:], bias_sb[:, m:m + 1])

            # avn is quantized to fp8 before the reshard; the collective
            # itself runs on a bf16 VIEW of those bytes ([512, 256] bf16 ==
            # [512, 512] fp8) — fp8-dtype collectives measured ~20x slower,
            # and this also removes any post-a2a convert from the tail.
            a2a_in = [dram.tile([512, 256], BF16, name=f"a2ain{hh}")
                      for hh in range(2)]
            a2a_out = [dram.tile([512, 256], BF16, name=f"a2aout{hh}")
                       for hh in range(2)]

            # tiny warmup collective: the first AllToAll of a NEFF pays a
            # large CC-stream start delay; paying it here (overlapped with
            # phase Q / the runtime barrier) takes it off the mid-kernel
            # critical path
            warm_in = dram.tile([8, 256], BF16, name="warm_in")
            warm_out = dram.tile([8, 256], BF16, name="warm_out")
            warm_sb = pp.tile([8, 256], BF16, name="warm_sb")
            nc.vector.memset(warm_sb[:], 0.0)
            nc.sync.dma_start(warm_in[:], warm_sb[:])
            nc.gpsimd.collective_compute(
                "AllToAll", mybir.AluOpType.bypass,
                replica_groups=groups,
                ins=[warm_in.opt()], outs=[warm_out.opt()])

            def qkv_xs(nch):
                xs_t = []
                for k in range(KT):
                    xs = xsp.tile([128, 512], BF16, name="xs", tag="xs")
                    nc.sync.dma_start(
                        xs[:], xT_ext[k * 128:(k + 1) * 128,
                                      nch * 512:(nch + 1) * 512])
                    xs_t.append(xs)
                return xs_t

            xs_cache = {0: xs0_t}

            def transp_j(j, tpool, ttag):
                """v transpose for key-tile j -> v_aug[h][j] (+ones col)."""
                tps = tpool.tile([128, 128], BF16, name="tps", tag=ttag)
                nc.tensor.transpose(
                    tps[:], vT[:, j * 128:(j + 1) * 128], ident[:])
                for h in range(2):
                    nc.vector.tensor_copy(
                        v_aug[h][j][:, 0:64], tps[:, h * 64:(h + 1) * 64])
                    nc.vector.memset(v_aug[h][j][:, 64:65], 1.0)

            # ------------- phase Q: full qkv for chunks 0-1 -------------
            with ExitStack() as qctx:
                qps = qctx.enter_context(
                    tc.tile_pool(name="qkv_ps", bufs=1, space="PSUM"))
                trp = qctx.enter_context(
                    tc.tile_pool(name="tr_ps", bufs=2, space="PSUM"))
                for nch in range(2):
                    if nch not in xs_cache:
                        xs_cache[nch] = qkv_xs(nch)
                    xs_t = xs_cache[nch]
                    psums = [qps.tile([128, 512], F32, name=f"qps{m}",
                                      tag=f"qps{m}") for m in range(3)]
                    for k in range(KT):
                        for m in range(3):
                            nc.tensor.matmul(
                                psums[m][:],
                                wq_sb[k][:, m * 128:(m + 1) * 128],
                                xs_t[k][:], start=(k == 0), stop=(k == KT - 1))
                    for m in range(3):
                        nc.vector.tensor_copy(
                            mtiles[m][:, nch * 512:(nch + 1) * 512],
                            psums[m][:])
                    for j in range(4 * nch, 4 * nch + 4):
                        transp_j(j, trp, "tps")

            # ---------------- phase A: attention + fillers ----------------
            with ExitStack() as actx:
                expp = actx.enter_context(tc.tile_pool(name="exp", bufs=4))
                avup = actx.enter_context(tc.tile_pool(name="avu", bufs=3))
                normp = actx.enter_context(tc.tile_pool(name="norm", bufs=2))
                avnp = actx.enter_context(tc.tile_pool(name="avn", bufs=2))
                wpp = actx.enter_context(tc.tile_pool(name="wp", bufs=1))
                wp8p = actx.enter_context(tc.tile_pool(name="wp8", bufs=1))
                rhs8p = actx.enter_context(tc.tile_pool(name="rhs8", bufs=1))
                sps = actx.enter_context(
                    tc.tile_pool(name="scores_ps", bufs=2, space="PSUM"))
                avps = actx.enter_context(
                    tc.tile_pool(name="av_ps", bufs=1, space="PSUM"))
                bcps = actx.enter_context(
                    tc.tile_pool(name="bc_ps", bufs=1, space="PSUM"))
                pjps = actx.enter_context(
                    tc.tile_pool(name="pj_ps", bufs=1, space="PSUM"))

                # tail prefetch rides the scalar HWDGE queue: its ~590ns/
                # issue cost drains during phase Q (before the first exp),
                # and the sync queue stays clear for the filler xs loads.
                fT_sb = []
                for t in range(KT):
                    ft = pp.tile([128, 512], BF16, name=f"fTsb{t}")
                    nc.scalar.dma_start(ft[:], fT_ext[t * 128:(t + 1) * 128, :])
                    fT_sb.append(ft)
                wp_sb = []
                for t in range(KT):
                    w = wpp.tile([128, C], BF16, name=f"wp_{t}")
                    nc.scalar.dma_start(w[:], wproj_ext[t * 128:(t + 1) * 128, :])
                    wp_sb.append(w)
                wp8_sb = []
                for hh in range(2):
                    w8 = wp8p.tile([128, 4, C], F8, name=f"wp8_{hh}")
                    nc.scalar.dma_start(w8[:], wp8_ext[hh][:])
                    wp8_sb.append(w8)

                pe_work = []   # 4-m-tile DoubleRow units for the h0 half
                rhs8_sb = {}

                # ---- fine-grained PE filler stream -----------------------
                # Yields callables; each emits ONE PE instruction (plus a
                # trailing copy on gpsimd when a pass completes, so the DVE
                # queue never gates the kT/qT/vT availability).  Order is
                # deadline-driven for the b-outer chunk sequence:
                #   ci0 (h0,b0,c0): kT/v ch2 by kj8, kT/v ch3 by kj12,
                #                   qT ch2,3 by end of ci0 (ci1 reads them)
                #   ci4+ (b1): everything of ch4-7 — 3 chunks of slack
                def filler_gen():
                    # per chunk the LAST pass releases its xs tiles; for
                    # ch2/3 the k and v passes lead (scores/av of ci0 kj8+
                    # consume them) and q trails (first read at ci1)
                    order = [(2, 0), (2, 2), (3, 0), (3, 2), (2, 1), (3, 1),
                             (4, 0), (4, 1), (4, 2), (5, 0), (5, 1), (5, 2),
                             (6, 0), (6, 1), (6, 2), (7, 0), (7, 1), (7, 2)]
                    last_m = {2: 1, 3: 1, 4: 2, 5: 2, 6: 2, 7: 2}
                    for nch, m in order:
                        if nch not in xs_cache:
                            xs_cache[nch] = qkv_xs(nch)
                        xs_t = xs_cache[nch]
                        pjt = pjps.tile([128, 512], F32, name="qkvd",
                                        tag="pj")
                        for k in range(KT):
                            yield lambda k=k, m=m, pjt=pjt, xs_t=xs_t: \
                                nc.tensor.matmul(
                                    pjt[:],
                                    wq_sb[k][:, m * 128:(m + 1) * 128],
                                    xs_t[k][:], start=(k == 0),
                                    stop=(k == KT - 1))
                        def finish(nch=nch, m=m, pjt=pjt):
                            nc.vector.tensor_copy(
                                mtiles[m][:, nch * 512:(nch + 1) * 512],
                                pjt[:])
                            if m == last_m[nch]:
                                del xs_cache[nch]
                        yield finish
                        if m == 2:
                            for j in range(4 * nch, 4 * nch + 4):
                                yield lambda j=j: transp_j(j, bcps, "bc")

                fillers = filler_gen()
                fillers_done = [False]

                def fill(nmm):
                    for _ in range(nmm):
                        step = next(fillers, None)
                        if step is None:
                            fillers_done[0] = True
                            return
                        step()

                def load_rhs8(hh):
                    """Stack the 8 received [64,512] bf16 tiles of half hh
                    into two DoubleRow rhs tiles [128, 2, 512] (bf16 view,
                    bytes are already fp8).  These DMAs carry a wait on the
                    a2a output; they ride the gpsimd SWDGE queue, where the
                    only thing they can park is a later collective trigger
                    (which waits on the same a2a anyway).  On a HWDGE queue
                    the list scheduler hoists them ahead of mid-loop
                    staging/attn_t DMAs and parks the whole queue on the
                    collective semaphore (measured: a 15us ci7 stall)."""
                    tiles = []
                    for u in range(2):
                        r = rhs8p.tile([128, 2, 512], F8, name=f"r8_{hh}{u}")
                        for i in range(2):
                            s = 2 * u + i
                            if hh == 0:
                                eng = nc.gpsimd
                            else:
                                # final half: parking the HWDGE queues is
                                # free (everything behind these loads also
                                # waits on the a2a), and HWDGE issue is
                                # faster than SWDGE on the critical tail
                                eng = nc.sync if i == 0 else nc.scalar
                            eng.dma_start(
                                r[:, i, :].bitcast(BF16),
                                a2a_out[hh][s * 128:(s + 1) * 128, :])
                        tiles.append(r)
                    rhs8_sb[hh] = tiles

                def proj8_unit(hh, ms, final):
                    """DoubleRow avn@W for head-parity half hh, m-tiles ms."""
                    if hh not in rhs8_sb:
                        load_rhs8(hh)
                    r8 = rhs8_sb[hh]
                    w8 = wp8_sb[hh]
                    for m in ms:
                        pj = pjps.tile([128, 512], F32, name="pj8", tag="pj")
                        for u in range(2):
                            nc.tensor.matmul(
                                pj[:], w8[:, 2 * u:2 * u + 2,
                                          m * 128:(m + 1) * 128],
                                r8[u][:], start=(u == 0), stop=(u == 1),
                                perf_mode=DR)
                        if final:
                            ot = avnp.tile([128, 512], BF16, name="ot",
                                           tag="ot")
                            nc.vector.tensor_tensor(
                                ot[:], pj[:], out_acc[m][:], ADD)
                            eng = nc.sync if m % 2 == 0 else nc.scalar
                            eng.dma_start(
                                out_t_ext[m * 128:(m + 1) * 128, :], ot[:])
                        else:
                            nc.vector.tensor_tensor(
                                out_acc[m][:], pj[:], out_acc[m][:], ADD)

                def fw_tail():
                    """f @ W_proj: dependency-free bf16 matmuls, emitted in
                    the tail to cover the final AllToAll window (real work
                    instead of burn matmuls; keeps the HAM governor warm
                    for the DoubleRow contraction that follows)."""
                    for m in range(8):
                        pj = pjps.tile([128, 512], F32, name="pjf", tag="pj")
                        for k in range(KT):
                            nc.tensor.matmul(
                                pj[:], wp_sb[k][:, m * 128:(m + 1) * 128],
                                fT_sb[k][:], start=(k == 0),
                                stop=(k == KT - 1))
                        nc.vector.tensor_tensor(
                            out_acc[m][:], pj[:], out_acc[m][:], ADD)

                def norm_pre(avu):
                    """1/denom chain — latency starts at chunk end.  The
                    reciprocal runs in a [128, 8] spread of the denominator
                    row (DVE cost is free-size only: 8 cycles, not 1024);
                    both DMAs use the same p-major element order.  (A
                    direct [1,1024] reciprocal_approx_fast produced garbage
                    on hardware — the custom-DVE ucode appears to assume a
                    multi-partition layout — while simulating correctly.)"""
                    dn = normp.tile([128, 8], F32, name="dn", tag="dn")
                    nc.sync.dma_start(dn[:], avu[64:65, :])
                    dninv = normp.tile([128, 8], F32, name="dninv",
                                       tag="dninv")
                    nc.vector.reciprocal_approx_fast(dninv[:], dn[:])
                    dnb = normp.tile([128, 8], BF16, name="dnb", tag="dnb")
                    # on DVE, not ACT: an ACT copy would make every next
                    # chunk's exps queue behind this chain (ACT is in-order)
                    nc.vector.tensor_copy(dnb[:], dninv[:])
                    dinvb = normp.tile([1, 1024], BF16, name="dinvb",
                                       tag="dinvb")
                    nc.sync.dma_start(dinvb[:], dnb[:])
                    return dinvb

                def norm_chunk(h, b, ch, avu, dinvb):
                    """avn32 = 32*avu[0:64]/avu[64]; attn_t out (x32, the
                    host divides); bf16 a2a staging (x32 by design)."""
                    po = h * 64
                    cs = b * 2048 + ch * 1024
                    avn32 = avnp.tile([64, 1024], BF16, name="avn32",
                                      tag="avn32")
                    avn8 = avnp.tile([64, 1024], F8, name="avn8", tag="avn8")
                    for s in range(2):
                        bc = bcps.tile([64, 512], F32, name="bc", tag="bc")
                        nc.tensor.matmul(bc[:], ones64b[:],
                                         dinvb[:, s * 512:(s + 1) * 512],
                                         start=True, stop=True)
                        # fp8 staging product first: the collective waits
                        # on it, the bf16 attn copy can trail
                        nc.vector.tensor_tensor(
                            avn8[:, s * 512:(s + 1) * 512],
                            avu[0:64, s * 512:(s + 1) * 512],
                            bc[:], MULT)
                        nc.vector.tensor_tensor(
                            avn32[:, s * 512:(s + 1) * 512],
                            avu[0:64, s * 512:(s + 1) * 512],
                            bc[:], MULT)
                    for j in (cs // 512, cs // 512 + 1):
                        off = j * 512 - cs
                        nc.sync.dma_start(
                            a2a_in[h][j * 64:(j + 1) * 64, :],
                            avn8[:, off:off + 512].bitcast(BF16))
                    nc.sync.dma_start(
                        attn_t_ext[po:po + 64, cs:cs + 1024], avn32[:])

                def mm1_kj(h, b, cs, kj, scores_q):
                    po = h * 64
                    jt = b * 16 + kj
                    sc = sps.tile([128, 1024], F32, name="scores", tag="sc")
                    for s in range(2):
                        nc.tensor.matmul(
                            sc[:, s * 512:(s + 1) * 512],
                            kT[po:po + 64, jt * 128:(jt + 1) * 128],
                            qT[po:po + 64, cs + s * 512:cs + (s + 1) * 512],
                            start=True, stop=True)
                    scores_q[kj] = sc

                def fire_a2a(h):
                    nc.gpsimd.collective_compute(
                        "AllToAll", mybir.AluOpType.bypass,
                        replica_groups=groups,
                        ins=[a2a_in[h].opt()], outs=[a2a_out[h].opt()])
                    if h == 0:
                        pe_work.extend([(0, range(0, 4)), (0, range(4, 8))])

                def do_chunk(ci, h, b, ch, pending):
                    cs = b * 2048 + ch * 1024
                    av = avps.tile([128, 1024], F32, name="av", tag="av")
                    scores_q = {}
                    mm1_kj(h, b, cs, 0, scores_q)
                    for kj in range(16):
                        if kj + 1 < 16:
                            mm1_kj(h, b, cs, kj + 1, scores_q)
                        if kj == 4 and pending is not None:
                            pci = pending[0]
                            norm_chunk(*pending[1:])
                            pending = None
                            if pci == 5:
                                # h0's 4th chunk is normed: its a2a can go
                                fire_a2a(0)
                        if not fillers_done[0]:
                            fill(4 if ci == 0 else (3 if ci < 4 else 1))
                        sc = scores_q.pop(kj)
                        ex = expp.tile([128, 1024], BF16, name="ex", tag="ex")
                        nc.scalar.activation(ex[:], sc[:], EXP, scale=SCALE)
                        jt = b * 16 + kj
                        for s in range(2):
                            nc.tensor.matmul(
                                av[0:65, s * 512:(s + 1) * 512],
                                v_aug[h][jt][:],
                                ex[:, s * 512:(s + 1) * 512],
                                start=(kj == 0), stop=(kj == 15))
                    avu = avup.tile([65, 1024], F32, name="avu", tag="avu")
                    nc.vector.tensor_copy(avu[:], av[0:65, :])
                    return (ci, h, b, ch, avu, norm_pre(avu))

                # batch-outer chunk order; h0 finishes at ci=5
                seq = [(0, 0, 0), (0, 0, 1), (1, 0, 0), (1, 0, 1),
                       (0, 1, 0), (0, 1, 1), (1, 1, 0), (1, 1, 1)]
                pending = None
                for ci, (h, b, ch) in enumerate(seq):
                    pending = do_chunk(ci, h, b, ch, pending)
                # tail: drain leftover fillers, last chunk's norm, fire the
                # final collective, cover its transfer with f@W + the h0
                # DoubleRow units, then the h1 contraction + output stores.
                fill(1 << 30)
                norm_chunk(*pending[1:])
                fire_a2a(1)
                fw_tail()
                while pe_work:
                    proj8_unit(*pe_work.pop(0), final=False)
                proj8_unit(1, range(8), final=True)

    nc.compile()
    return nc


def kernel(x, f, W_qkv, W_proj, b_proj):
    import ml_dtypes
    from concourse.bass_utils import run_bass_kernel_spmd

    global _cached
    if _cached is None:
        _cached = _build()
    nc = _cached

    BF = ml_dtypes.bfloat16
    F8 = ml_dtypes.float8_e4m3
    x = np.ascontiguousarray(np.asarray(x, dtype=np.float32))
    f = np.ascontiguousarray(np.asarray(f, dtype=np.float32))
    W_qkv = np.asarray(W_qkv, dtype=np.float32)
    W_proj = np.asarray(W_proj, dtype=np.float32)
    b_proj = np.asarray(b_proj, dtype=np.float32)

    Wq, Wk, Wv = W_qkv[:, 0:C], W_qkv[:, C:2 * C], W_qkv[:, 2 * C:3 * C]
    xT = np.ascontiguousarray(x.reshape(BN, C).T.astype(BF))
    fT = (f.reshape(BN, C).T * PS).astype(BF)
    wproj_b = np.ascontiguousarray((W_proj * PS).astype(BF))
    bprojT = np.ascontiguousarray((b_proj * PS * PS).reshape(8, 128).T)

    # DoubleRow fp8 W_proj halves: wp8_h[p, s, m] = (W_proj*PS)[row, m] with
    # row = head*64 + p%64, head = 4s + h + 2*(p//64)
    Wp32 = W_proj * PS
    p = np.arange(128)
    wp8 = []
    for h in range(2):
        rows = np.empty((128, 4), np.int64)
        for s in range(4):
            head = 4 * s + h + 2 * (p // 64)
            rows[:, s] = head * 64 + (p % 64)
        wp8.append(np.ascontiguousarray(
            Wp32[rows, :].astype(F8).reshape(128, 4 * C)))

    in_maps = []
    for c in range(N_CORES):
        cols = slice(c * 128, (c + 1) * 128)     # heads 2c, 2c+1
        wkqv = np.ascontiguousarray(np.concatenate(
            [Wk[:, cols], Wq[:, cols], Wv[:, cols]], axis=1).astype(BF))
        in_maps.append({
            "xT": xT,
            "wkqv": wkqv,
            "fT": np.ascontiguousarray(fT[:, c * 512:(c + 1) * 512]),
            "wproj": wproj_b,
            "wp8_0": wp8[0],
            "wp8_1": wp8[1],
            "bprojT": bprojT,
        })

    res = run_bass_kernel_spmd(nc, in_maps, core_ids=list(range(N_CORES)))

    attn = np.empty((BN, C), dtype=np.float32)
    out = np.empty((BN, C), dtype=np.float32)
    for c in range(N_CORES):
        r = res.results[c]
        attn[:, c * 128:(c + 1) * 128] = r["attn_t"].T.astype(np.float32) / PS
        out[c * 512:(c + 1) * 512, :] = \
            r["out_t"].T.astype(np.float32) / (PS * PS)
    return out.reshape(B, N, C), attn.reshape(B, N, C)


# revision 23
# speedup vs baseline: 1.0605x; 1.0334x over previous
"""Distributed MHA kernel for Trainium2 (8 NeuronCores).

Problem: x,f:(2,2048,1024), W_qkv:(1024,3072), W_proj:(1024,1024), H=16 heads.
reference returns (out, attn2gcn) with
  attn2gcn = softmax(q k^T / sqrt(64)) v   (per head, concat over heads)
  out      = (attn2gcn + f) @ W_proj + b_proj

Sharding: tensor-parallel over heads — core c owns heads 2c, 2c+1 for both
batches (column block c*128 of the hidden dim).  Attention arithmetic is
bf16 matmuls with fp32 PSUM (the attn output's max-abs rel-err budget is
too tight for fp8 anywhere on that path: ex/v at e4m3 alone would cost
~1.7e-2 of the 2e-2 budget); softmax stays fp32 on ACT.

Timeline model (v2): the attention phase is a balanced PE/ACT race — ACT
exp costs (1024+352)/1.2GHz = 1113ns per kj and the core PE work (2 scores
+ 2 av matmuls, F=512) is ~1050ns/kj, so every extra PE cycle in the
window extends the kernel ~1:1.  Design consequences:
  - chunk order is batch-outer (b0: h0c0,h0c1,h1c0,h1c1; then b1) so the
    deferred qkv for batch 1 has 4 chunks of slack; only chunks 0-1 run
    eagerly in phase Q.  Deferred qkv is injected ONE matmul at a time
    (not 8-matmul bursts) between the ACT-feeding mm1 and the av matmuls.
  - h0's AllToAll fires after its last chunk (ci=5, during ci=6); h1's
    fires post-loop, its transfer covered by the f@W matmuls (real work
    that replaces the old burn matmuls and keeps the HAM clock warm).
  - out_acc[m] is bias-initialized at t0 so the f@W and h0-DoubleRow
    accumulations commute (plain DVE adds, any order).
  - DMA issue costs ~590ns ON THE ISSUING ENGINE'S QUEUE: the scalar
    (ACT) HWDGE queue carries only pre-attention traffic (weights, tail
    prefetch — its issue cost drains during phase Q) plus post-attention
    stores; xs/staging/attn_t/norm ride sync; the a2a-gated rhs8 loads
    ride gpsimd SWDGE (on a HWDGE queue the list scheduler hoists them
    and parks the queue on the collective semaphore).
  - the ACT exp table load (~1.3us) is prepaid with a dummy exp at t0.

The projection is split: out = f @ W_proj + attn2gcn @ W_proj + b.
  - attn2gcn (avn) rides the AllToAll as bf16 (x32; fp8 collectives
    measured pathologically slow), is cast to fp8e4 after the reshard,
    and the contraction runs as fp8 DoubleRow matmuls.
  - scales: f, W_proj staged x32 (bf16), avn x32 (fp8) -> psum carries
    1024*out; the host divides by 1024 after gathering (pure numpy).
"""

import numpy as np

B, N, C, H, D = 2, 2048, 1024, 16, 64
BN = B * N
SCALE = D ** -0.5
N_CORES = 8
KT = C // 128      # 8 contraction tiles
NCH = BN // 512    # 8 qkv free chunks
PS = 32.0          # fp8/bf16 staging scale for the projection operands

_cached = None


def _build():
    from contextlib import ExitStack

    import concourse.mybir as mybir
    import concourse.tile as tile
    from concourse import bacc
    from concourse.masks import make_identity

    F32 = mybir.dt.float32
    BF16 = mybir.dt.bfloat16
    F8 = mybir.dt.float8e4
    EXP = mybir.ActivationFunctionType.Exp
    DR = mybir.MatmulPerfMode.DoubleRow
    ADD = mybir.AluOpType.add
    MULT = mybir.AluOpType.mult

    nc = bacc.Bacc("TRN2", target_bir_lowering=False, debug=False,
                   num_devices=N_CORES)

    xT_ext = nc.dram_tensor("xT", [C, BN], BF16, kind="ExternalInput").ap()
    wkqv_ext = nc.dram_tensor("wkqv", [C, 384], BF16, kind="ExternalInput").ap()
    fT_ext = nc.dram_tensor("fT", [C, 512], BF16, kind="ExternalInput").ap()
    wproj_ext = nc.dram_tensor("wproj", [C, C], BF16, kind="ExternalInput").ap()
    wp8_ext = [nc.dram_tensor(f"wp8_{h}", [128, 4 * C], mybir.dt.float8e4,
                              kind="ExternalInput").ap() for h in range(2)]
    bprojT_ext = nc.dram_tensor("bprojT", [128, 8], F32, kind="ExternalInput").ap()
    attn_t_ext = nc.dram_tensor("attn_t", [128, BN], BF16, kind="ExternalOutput").ap()
    out_t_ext = nc.dram_tensor("out_t", [C, 512], BF16, kind="ExternalOutput").ap()

    groups = [list(range(N_CORES))]

    with tile.TileContext(nc) as tc:
        with ExitStack() as octx:
            pp = octx.enter_context(tc.tile_pool(name="persist", bufs=1))
            kqp = octx.enter_context(tc.tile_pool(name="kq", bufs=1))
            vap = octx.enter_context(tc.tile_pool(name="vaug", bufs=1))
            vtp = octx.enter_context(tc.tile_pool(name="vt", bufs=1))
            wqp = octx.enter_context(tc.tile_pool(name="wq", bufs=1))
            xsp = octx.enter_context(tc.tile_pool(name="xs", bufs=16))
            oaccp = octx.enter_context(tc.tile_pool(name="oacc", bufs=1))
            dram = octx.enter_context(
                tc.tile_pool(name="dram", bufs=1, space="DRAM"))
            # startup DMA: interleave qkv weights and the first TWO x chunks
            # across both HWDGE queues so matmul k of phase Q has (wq[k],
            # xs0[k]) after ~2 transfers per queue.  sync: xs0-even + all of
            # xs1; scalar: wq (interleaved with xs0-odd) then tail prefetch.
            wq_sb = []
            xs0_t = []
            for k in range(KT):
                w = wqp.tile([128, 384], BF16, name=f"wq{k}")
                nc.scalar.dma_start(w[:], wkqv_ext[k * 128:(k + 1) * 128, :])
                wq_sb.append(w)
                xs = xsp.tile([128, 512], BF16, name="xs", tag="xs")
                eng = nc.sync if k % 2 == 0 else nc.scalar
                eng.dma_start(xs[:], xT_ext[k * 128:(k + 1) * 128, 0:512])
                xs0_t.append(xs)
            xs1_t = []
            for k in range(KT):
                xs = xsp.tile([128, 512], BF16, name="xs", tag="xs")
                nc.sync.dma_start(
                    xs[:], xT_ext[k * 128:(k + 1) * 128, 512:1024])
                xs1_t.append(xs)

            # small persistent scratch + warmup
            ident = pp.tile([128, 128], BF16)
            make_identity(nc, ident[:])
            # 32, not 1: the normalization broadcast then yields avn*32
            # directly (the x32 the a2a/proj stage wants); the host divides
            # the attn output by 32 after gathering.
            ones64b = pp.tile([1, 64], BF16)
            nc.vector.memset(ones64b[:], PS)
            # lhs for the last chunk's DMA-free norm: broadcasting the raw
            # denominator row with weight PS/1024 gives bc_d = d/32, whose
            # fast-reciprocal is directly the 32/d the avn products need
            ones64r = pp.tile([1, 64], BF16)
            nc.vector.memset(ones64r[:], PS / 1024.0)

            # prepay the ACT exp-table load (~1.3us) before the first real
            # exp; the scalar queue is idle during phase Q anyway
            dummy = pp.tile([8, 8], BF16, name="dummy")
            nc.vector.memset(dummy[:], 0.0)
            dummy2 = pp.tile([8, 8], BF16, name="dummy2")
            nc.scalar.activation(dummy2[:], dummy[:], EXP)

            # bias for the projection accumulators (tiny, scalar queue)
            bias_sb = pp.tile([128, 8], F32)
            nc.scalar.dma_start(bias_sb[:], bprojT_ext[:])

            kT = kqp.tile([128, BN], BF16, name="kT")
            qT = kqp.tile([128, BN], BF16, name="qT")
            vT = vtp.tile([128, BN], BF16, name="vT")
            mtiles = [kT, qT, vT]
            v_aug = [[vap.tile([128, 65], BF16, name=f"va{h}_{j}")
                      for j in range(32)] for h in range(2)]

            # out_acc[m] starts at b_proj*PS*PS; f@W and the h0 DoubleRow
            # passes then += into it in any order (DVE adds commute)
            out_acc = [oaccp.tile([128, 512], F32, name=f"oacc{m}")
                       for m in range(8)]
            for m in range(8):
                nc.vector.memset(out_acc[m][:], 0.0)
                nc.vector.tensor_scalar_add(
                    out_acc[m][:], out_acc[m][:], bias_sb[:, m:m + 1])

            # avn is quantized to fp8 before the reshard; the collective
            # itself runs on a bf16 VIEW of those bytes ([512, 256] bf16 ==
            # [512, 512] fp8) — fp8-dtype collectives measured ~20x slower,
            # and this also removes any post-a2a convert from the tail.
            a2a_in = [dram.tile([512, 256], BF16, name=f"a2ain{hh}")
                      for hh in range(2)]
            a2a_out = [dram.tile([512, 256], BF16, name=f"a2aout{hh}")
                       for hh in range(2)]

            # tiny warmup collective: the first AllToAll of a NEFF pays a
            # large CC-stream start delay; paying it here (overlapped with
            # phase Q / the runtime barrier) takes it off the mid-kernel
            # critical path
            warm_in = dram.tile([8, 256], BF16, name="warm_in")
            warm_out = dram.tile([8, 256], BF16, name="warm_out")
            warm_sb = pp.tile([8, 256], BF16, name="warm_sb")
            nc.vector.memset(warm_sb[:], 0.0)
            nc.sync.dma_start(warm_in[:], warm_sb[:])
            nc.gpsimd.collective_compute(
                "AllToAll", mybir.AluOpType.bypass,
                replica_groups=groups,
                ins=[warm_in.opt()], outs=[warm_out.opt()])

            def qkv_xs(nch):
                xs_t = []
                for k in range(KT):
                    xs = xsp.tile([128, 512], BF16, name="xs", tag="xs")
                    nc.sync.dma_start(
                        xs[:], xT_ext[k * 128:(k + 1) * 128,
                                      nch * 512:(nch + 1) * 512])
                    xs_t.append(xs)
                return xs_t

            xs_cache = {0: xs0_t, 1: xs1_t}

            def transp_j(j, tpool, ttag):
                """v transpose for key-tile j -> v_aug[h][j] (+ones col)."""
                tps = tpool.tile([128, 128], BF16, name="tps", tag=ttag)
                nc.tensor.transpose(
                    tps[:], vT[:, j * 128:(j + 1) * 128], ident[:])
                for h in range(2):
                    nc.vector.tensor_copy(
                        v_aug[h][j][:, 0:64], tps[:, h * 64:(h + 1) * 64])
                    nc.vector.memset(v_aug[h][j][:, 64:65], 1.0)

            # ------------- phase Q: full qkv for chunks 0-1 -------------
            with ExitStack() as qctx:
                qps = qctx.enter_context(
                    tc.tile_pool(name="qkv_ps", bufs=1, space="PSUM"))
                trp = qctx.enter_context(
                    tc.tile_pool(name="tr_ps", bufs=2, space="PSUM"))
                # HAM warm-up: ~64 tiny identity matmuls keep the PE busy
                # through the input DMA ramp so the clock governor reaches
                # full rate before the real phase-Q matmuls start (cold
                # phase Q measured ~2x slower: ~600ns per 512-col MM)
                for _ in range(48):
                    bp = trp.tile([128, 128], F32, name="burn", tag="burn")
                    nc.tensor.matmul(bp[:], ident[:], ident[:],
                                     start=True, stop=True)
                for nch in range(2):
                    if nch not in xs_cache:
                        xs_cache[nch] = qkv_xs(nch)
                    xs_t = xs_cache[nch]
                    psums = [qps.tile([128, 512], F32, name=f"qps{m}",
                                      tag=f"qps{m}") for m in range(3)]
                    for k in range(KT):
                        for m in range(3):
                            nc.tensor.matmul(
                                psums[m][:],
                                wq_sb[k][:, m * 128:(m + 1) * 128],
                                xs_t[k][:], start=(k == 0), stop=(k == KT - 1))
                    for m in range(3):
                        nc.vector.tensor_copy(
                            mtiles[m][:, nch * 512:(nch + 1) * 512],
                            psums[m][:])
                    for j in range(4 * nch, 4 * nch + 4):
                        transp_j(j, trp, "tps")

            # ---------------- phase A: attention + fillers ----------------
            with ExitStack() as actx:
                expp = actx.enter_context(tc.tile_pool(name="exp", bufs=4))
                avup = actx.enter_context(tc.tile_pool(name="avu", bufs=3))
                normp = actx.enter_context(tc.tile_pool(name="norm", bufs=2))
                avnp = actx.enter_context(tc.tile_pool(name="avn", bufs=2))
                wpp = actx.enter_context(tc.tile_pool(name="wp", bufs=1))
                wp8p = actx.enter_context(tc.tile_pool(name="wp8", bufs=1))
                rhs8p = actx.enter_context(tc.tile_pool(name="rhs8", bufs=1))
                sps = actx.enter_context(
                    tc.tile_pool(name="scores_ps", bufs=2, space="PSUM"))
                avps = actx.enter_context(
                    tc.tile_pool(name="av_ps", bufs=1, space="PSUM"))
                bcps = actx.enter_context(
                    tc.tile_pool(name="bc_ps", bufs=1, space="PSUM"))
                pjps = actx.enter_context(
                    tc.tile_pool(name="pj_ps", bufs=1, space="PSUM"))

                # tail prefetch rides the scalar HWDGE queue: its ~590ns/
                # issue cost drains during phase Q (before the first exp),
                # and the sync queue stays clear for the filler xs loads.
                fT_sb = []
                for t in range(KT):
                    ft = pp.tile([128, 512], BF16, name=f"fTsb{t}")
                    nc.scalar.dma_start(ft[:], fT_ext[t * 128:(t + 1) * 128, :])
                    fT_sb.append(ft)
                wp_sb = []
                for t in range(KT):
                    w = wpp.tile([128, C], BF16, name=f"wp_{t}")
                    nc.scalar.dma_start(w[:], wproj_ext[t * 128:(t + 1) * 128, :])
                    wp_sb.append(w)
                wp8_sb = []
                for hh in range(2):
                    w8 = wp8p.tile([128, 4, C], F8, name=f"wp8_{hh}")
                    nc.scalar.dma_start(w8[:], wp8_ext[hh][:])
                    wp8_sb.append(w8)

                pe_work = []   # 4-m-tile DoubleRow units for the h0 half
                rhs8_sb = {}

                # ---- fine-grained PE filler stream -----------------------
                # Yields callables; each emits ONE PE instruction (plus a
                # trailing copy on gpsimd when a pass completes, so the DVE
                # queue never gates the kT/qT/vT availability).  Order is
                # deadline-driven for the b-outer chunk sequence:
                #   ci0 (h0,b0,c0): kT/v ch2 by kj8, kT/v ch3 by kj12,
                #                   qT ch2,3 by end of ci0 (ci1 reads them)
                #   ci4+ (b1): everything of ch4-7 — 3 chunks of slack
                def filler_gen():
                    # per chunk the LAST pass releases its xs tiles; for
                    # ch2/3 the k and v passes lead (scores/av of ci0 kj8+
                    # consume them) and q trails (first read at ci1)
                    order = [(2, 0), (2, 2), (3, 0), (3, 2), (2, 1), (3, 1),
                             (4, 0), (4, 1), (4, 2), (5, 0), (5, 1), (5, 2),
                             (6, 0), (6, 1), (6, 2), (7, 0), (7, 1), (7, 2)]
                    last_m = {2: 1, 3: 1, 4: 2, 5: 2, 6: 2, 7: 2}
                    for nch, m in order:
                        if nch not in xs_cache:
                            xs_cache[nch] = qkv_xs(nch)
                        xs_t = xs_cache[nch]
                        pjt = pjps.tile([128, 512], F32, name="qkvd",
                                        tag="pj")
                        for k in range(KT):
                            yield lambda k=k, m=m, pjt=pjt, xs_t=xs_t: \
                                nc.tensor.matmul(
                                    pjt[:],
                                    wq_sb[k][:, m * 128:(m + 1) * 128],
                                    xs_t[k][:], start=(k == 0),
                                    stop=(k == KT - 1))
                        def finish(nch=nch, m=m, pjt=pjt):
                            nc.vector.tensor_copy(
                                mtiles[m][:, nch * 512:(nch + 1) * 512],
                                pjt[:])
                            if m == last_m[nch]:
                                del xs_cache[nch]
                        yield finish
                        if m == 2:
                            for j in range(4 * nch, 4 * nch + 4):
                                yield lambda j=j: transp_j(j, bcps, "bc")

                fillers = filler_gen()
                fillers_done = [False]

                def fill(nmm):
                    for _ in range(nmm):
                        step = next(fillers, None)
                        if step is None:
                            fillers_done[0] = True
                            return
                        step()

                def load_rhs8(hh):
                    """Stack the 8 received [64,512] bf16 tiles of half hh
                    into two DoubleRow rhs tiles [128, 2, 512] (bf16 view,
                    bytes are already fp8).  These DMAs carry a wait on the
                    a2a output; they ride the gpsimd SWDGE queue, where the
                    only thing they can park is a later collective trigger
                    (which waits on the same a2a anyway).  On a HWDGE queue
                    the list scheduler hoists them ahead of mid-loop
                    staging/attn_t DMAs and parks the whole queue on the
                    collective semaphore (measured: a 15us ci7 stall)."""
                    tiles = []
                    for u in range(2):
                        r = rhs8p.tile([128, 2, 512], F8, name=f"r8_{hh}{u}")
                        for i in range(2):
                            s = 2 * u + i
                            if hh == 0:
                                eng = nc.gpsimd
                            else:
                                # final half: parking the HWDGE queues is
                                # free (everything behind these loads also
                                # waits on the a2a), and HWDGE issue is
                                # faster than SWDGE on the critical tail
                                eng = nc.sync if i == 0 else nc.scalar
                            eng.dma_start(
                                r[:, i, :].bitcast(BF16),
                                a2a_out[hh][s * 128:(s + 1) * 128, :])
                        tiles.append(r)
                    rhs8_sb[hh] = tiles

                def proj8_unit(hh, ms, final):
                    """DoubleRow avn@W for head-parity half hh, m-tiles ms."""
                    if hh not in rhs8_sb:
                        load_rhs8(hh)
                    r8 = rhs8_sb[hh]
                    w8 = wp8_sb[hh]
                    for m in ms:
                        pj = pjps.tile([128, 512], F32, name="pj8", tag="pj")
                        for u in range(2):
                            nc.tensor.matmul(
                                pj[:], w8[:, 2 * u:2 * u + 2,
                                          m * 128:(m + 1) * 128],
                                r8[u][:], start=(u == 0), stop=(u == 1),
                                perf_mode=DR)
                        if final:
                            ot = avnp.tile([128, 512], BF16, name="ot",
                                           tag="ot")
                            nc.vector.tensor_tensor(
                                ot[:], pj[:], out_acc[m][:], ADD)
                            eng = nc.sync if m % 2 == 0 else nc.scalar
                            eng.dma_start(
                                out_t_ext[m * 128:(m + 1) * 128, :], ot[:])
                        else:
                            nc.vector.tensor_tensor(
                                out_acc[m][:], pj[:], out_acc[m][:], ADD)

                def fw_tail():
                    """f @ W_proj: dependency-free bf16 matmuls, emitted in
                    the tail to cover the final AllToAll window (real work
                    instead of burn matmuls; keeps the HAM governor warm
                    for the DoubleRow contraction that follows)."""
                    for m in range(8):
                        pj = pjps.tile([128, 512], F32, name="pjf", tag="pj")
                        for k in range(KT):
                            nc.tensor.matmul(
                                pj[:], wp_sb[k][:, m * 128:(m + 1) * 128],
                                fT_sb[k][:], start=(k == 0),
                                stop=(k == KT - 1))
                        nc.vector.tensor_tensor(
                            out_acc[m][:], pj[:], out_acc[m][:], ADD)

                def norm_pre(avu):
                    """1/denom chain — latency starts at chunk end.  The
                    reciprocal runs in a [128, 8] spread of the denominator
                    row (DVE cost is free-size only: 8 cycles, not 1024);
                    both DMAs use the same p-major element order.  (A
                    direct [1,1024] reciprocal_approx_fast produced garbage
                    on hardware — the custom-DVE ucode appears to assume a
                    multi-partition layout — while simulating correctly.)"""
                    dn = normp.tile([128, 8], F32, name="dn", tag="dn")
                    nc.sync.dma_start(dn[:], avu[64:65, :])
                    dninv = normp.tile([128, 8], F32, name="dninv",
                                       tag="dninv")
                    nc.vector.reciprocal_approx_fast(dninv[:], dn[:])
                    dnb = normp.tile([128, 8], BF16, name="dnb", tag="dnb")
                    # on DVE, not ACT: an ACT copy would make every next
                    # chunk's exps queue behind this chain (ACT is in-order)
                    nc.vector.tensor_copy(dnb[:], dninv[:])
                    dinvb = normp.tile([1, 1024], BF16, name="dinvb",
                                       tag="dinvb")
                    nc.sync.dma_start(dinvb[:], dnb[:])
                    return dinvb

                def norm_chunk(h, b, ch, avu, dinvb, last=False):
                    """avn32 = 32*avu[0:64]/avu[64]; attn_t out (x32, the
                    host divides); bf16 a2a staging (x32 by design).

                    last=True: the final chunk gates the final AllToAll, so
                    its chain must not touch the sync HWDGE queue — the list
                    scheduler's cost model runs ~40us optimistic by then and
                    parks the queue on any DMA it hoists (measured 45us).
                    Instead: broadcast the RAW denominator row via ones64r
                    (scale-folded), fast-reciprocal on the [64,512]
                    broadcast (multi-partition, same shape class as the
                    proven path), and stage via gpsimd SWDGE, where any
                    parking resolves at a2a(0)-done anyway."""
                    po = h * 64
                    cs = b * 2048 + ch * 1024
                    avn32 = avnp.tile([64, 1024], BF16, name="avn32",
                                      tag="avn32")
                    avn8 = avnp.tile([64, 1024], F8, name="avn8", tag="avn8")
                    if last:
                        drow = normp.tile([1, 1024], BF16, name="drow",
                                          tag="drow")
                        nc.vector.tensor_copy(drow[:], avu[64:65, :])
                    for s in range(2):
                        bc = bcps.tile([64, 512], F32, name="bc", tag="bc")
                        if last:
                            nc.tensor.matmul(bc[:], ones64r[:],
                                             drow[:, s * 512:(s + 1) * 512],
                                             start=True, stop=True)
                            rcp = normp.tile([64, 512], F32, name="rcp",
                                             tag="rcp")
                            nc.vector.reciprocal_approx_fast(rcp[:], bc[:])
                            scl = rcp
                        else:
                            nc.tensor.matmul(bc[:], ones64b[:],
                                             dinvb[:, s * 512:(s + 1) * 512],
                                             start=True, stop=True)
                            scl = bc
                        # fp8 staging product first: the collective waits
                        # on it, the bf16 attn copy can trail
                        nc.vector.tensor_tensor(
                            avn8[:, s * 512:(s + 1) * 512],
                            avu[0:64, s * 512:(s + 1) * 512],
                            scl[:], MULT)
                        nc.vector.tensor_tensor(
                            avn32[:, s * 512:(s + 1) * 512],
                            avu[0:64, s * 512:(s + 1) * 512],
                            scl[:], MULT)
                    st_eng = nc.gpsimd if last else nc.sync
                    for j in (cs // 512, cs // 512 + 1):
                        off = j * 512 - cs
                        st_eng.dma_start(
                            a2a_in[h][j * 64:(j + 1) * 64, :],
                            avn8[:, off:off + 512].bitcast(BF16))
                    st_eng.dma_start(
                        attn_t_ext[po:po + 64, cs:cs + 1024], avn32[:])

                def mm1_kj(h, b, cs, kj, scores_q):
                    po = h * 64
                    jt = b * 16 + kj
                    sc = sps.tile([128, 1024], F32, name="scores", tag="sc")
                    for s in range(2):
                        nc.tensor.matmul(
                            sc[:, s * 512:(s + 1) * 512],
                            kT[po:po + 64, jt * 128:(jt + 1) * 128],
                            qT[po:po + 64, cs + s * 512:cs + (s + 1) * 512],
                            start=True, stop=True)
                    scores_q[kj] = sc

                def fire_a2a(h):
                    nc.gpsimd.collective_compute(
                        "AllToAll", mybir.AluOpType.bypass,
                        replica_groups=groups,
                        ins=[a2a_in[h].opt()], outs=[a2a_out[h].opt()])
                    if h == 0:
                        pe_work.extend([(0, range(0, 4)), (0, range(4, 8))])

                def do_chunk(ci, h, b, ch, pending):
                    cs = b * 2048 + ch * 1024
                    av = avps.tile([128, 1024], F32, name="av", tag="av")
                    scores_q = {}
                    mm1_kj(h, b, cs, 0, scores_q)
                    for kj in range(16):
                        if kj + 1 < 16:
                            mm1_kj(h, b, cs, kj + 1, scores_q)
                        if kj == 4 and pending is not None:
                            pci = pending[0]
                            norm_chunk(*pending[1:])
                            pending = None
                            if pci == 5:
                                # h0's 4th chunk is normed: its a2a can go
                                fire_a2a(0)
                        if not fillers_done[0]:
                            fill(4 if ci == 0 else (3 if ci < 4 else 1))
                        sc = scores_q.pop(kj)
                        ex = expp.tile([128, 1024], BF16, name="ex", tag="ex")
                        nc.scalar.activation(ex[:], sc[:], EXP, scale=SCALE)
                        jt = b * 16 + kj
                        for s in range(2):
                            nc.tensor.matmul(
                                av[0:65, s * 512:(s + 1) * 512],
                                v_aug[h][jt][:],
                                ex[:, s * 512:(s + 1) * 512],
                                start=(kj == 0), stop=(kj == 15))
                    avu = avup.tile([65, 1024], F32, name="avu", tag="avu")
                    nc.vector.tensor_copy(avu[:], av[0:65, :])
                    # the last chunk normalizes via the DMA-free path
                    dinvb = None if ci == 7 else norm_pre(avu)
                    return (ci, h, b, ch, avu, dinvb)

                # batch-outer chunk order; h0 finishes at ci=5
                seq = [(0, 0, 0), (0, 0, 1), (1, 0, 0), (1, 0, 1),
                       (0, 1, 0), (0, 1, 1), (1, 1, 0), (1, 1, 1)]
                pending = None
                for ci, (h, b, ch) in enumerate(seq):
                    pending = do_chunk(ci, h, b, ch, pending)
                # tail: drain leftover fillers, last chunk's norm, fire the
                # final collective, cover its transfer with f@W + the h0
                # DoubleRow units, then the h1 contraction + output stores.
                # ORDER MATTERS: fire_a2a(1) must be emitted BEFORE anything
                # gated on a2a(0) (rhs8 loads / DoubleRow-h0) — the list
                # scheduler otherwise queues the a2a(1) trigger behind them
                # on gpsimd and delays it until a2a(0) completes (~mid-ci7
                # under the batch-outer chunk order; measured +13us).
                fill(1 << 30)
                norm_chunk(*pending[1:], last=True)
                fire_a2a(1)
                fw_tail()
                while pe_work:
                    proj8_unit(*pe_work.pop(0), final=False)
                proj8_unit(1, range(8), final=True)

    nc.compile()
    return nc


def kernel(x, f, W_qkv, W_proj, b_proj):
    import ml_dtypes
    from concourse.bass_utils import run_bass_kernel_spmd

    global _cached
    if _cached is None:
        _cached = _build()
    nc = _cached

    BF = ml_dtypes.bfloat16
    F8 = ml_dtypes.float8_e4m3
    x = np.ascontiguousarray(np.asarray(x, dtype=np.float32))
    f = np.ascontiguousarray(np.asarray(f, dtype=np.float32))
    W_qkv = np.asarray(W_qkv, dtype=np.float32)
    W_proj = np.asarray(W_proj, dtype=np.float32)
    b_proj = np.asarray(b_proj, dtype=np.float32)

    Wq, Wk, Wv = W_qkv[:, 0:C], W_qkv[:, C:2 * C], W_qkv[:, 2 * C:3 * C]
    xT = np.ascontiguousarray(x.reshape(BN, C).T.astype(BF))
    fT = (f.reshape(BN, C).T * PS).astype(BF)
    wproj_b = np.ascontiguousarray((W_proj * PS).astype(BF))
    bprojT = np.ascontiguousarray((b_proj * PS * PS).reshape(8, 128).T)

    # DoubleRow fp8 W_proj halves: wp8_h[p, s, m] = (W_proj*PS)[row, m] with
    # row = head*64 + p%64, head = 4s + h + 2*(p//64)
    Wp32 = W_proj * PS
    p = np.arange(128)
    wp8 = []
    for h in range(2):
        rows = np.empty((128, 4), np.int64)
        for s in range(4):
            head = 4 * s + h + 2 * (p // 64)
            rows[:, s] = head * 64 + (p % 64)
        wp8.append(np.ascontiguousarray(
            Wp32[rows, :].astype(F8).reshape(128, 4 * C)))

    in_maps = []
    for c in range(N_CORES):
        cols = slice(c * 128, (c + 1) * 128)     # heads 2c, 2c+1
        wkqv = np.ascontiguousarray(np.concatenate(
            [Wk[:, cols], Wq[:, cols], Wv[:, cols]], axis=1).astype(BF))
        in_maps.append({
            "xT": xT,
            "wkqv": wkqv,
            "fT": np.ascontiguousarray(fT[:, c * 512:(c + 1) * 512]),
            "wproj": wproj_b,
            "wp8_0": wp8[0],
            "wp8_1": wp8[1],
            "bprojT": bprojT,
        })

    res = run_bass_kernel_spmd(nc, in_maps, core_ids=list(range(N_CORES)))

    attn = np.empty((BN, C), dtype=np.float32)
    out = np.empty((BN, C), dtype=np.float32)
    for c in range(N_CORES):
        r = res.results[c]
        attn[:, c * 128:(c + 1) * 128] = r["attn_t"].T.astype(np.float32) / PS
        out[c * 512:(c + 1) * 512, :] = \
            r["out_t"].T.astype(np.float32) / (PS * PS)
    return out.reshape(B, N, C), attn.reshape(B, N, C)


# revision 34
# speedup vs baseline: 1.1278x; 1.0634x over previous
"""Distributed MHA kernel for Trainium2 (8 NeuronCores).

Problem: x,f:(2,2048,1024), W_qkv:(1024,3072), W_proj:(1024,1024), H=16 heads.
reference returns (out, attn2gcn) with
  attn2gcn = softmax(q k^T / sqrt(64)) v   (per head, concat over heads)
  out      = (attn2gcn + f) @ W_proj + b_proj

Sharding: tensor-parallel over heads — core c owns heads 2c, 2c+1 for both
batches (column block c*128 of the hidden dim).  Attention arithmetic is
bf16 matmuls with fp32 PSUM (the attn output's max-abs rel-err budget is
too tight for fp8 anywhere on that path: ex/v at e4m3 alone would cost
~1.7e-2 of the 2e-2 budget); softmax stays fp32 on ACT.

Timeline model (v2): the attention phase is a balanced PE/ACT race — ACT
exp costs (1024+352)/1.2GHz = 1113ns per kj and the core PE work (2 scores
+ 2 av matmuls, F=512) is ~1050ns/kj, so every extra PE cycle in the
window extends the kernel ~1:1.  Design consequences:
  - chunk order is batch-outer (b0: h0c0,h0c1,h1c0,h1c1; then b1) so the
    deferred qkv for batch 1 has 4 chunks of slack; only chunks 0-1 run
    eagerly in phase Q.  Deferred qkv is injected ONE matmul at a time
    (not 8-matmul bursts) between the ACT-feeding mm1 and the av matmuls.
  - h0's AllToAll fires after its last chunk (ci=5, during ci=6); h1's
    fires post-loop, its transfer covered by the f@W matmuls (real work
    that replaces the old burn matmuls and keeps the HAM clock warm).
  - out_acc[m] is bias-initialized at t0 so the f@W and h0-DoubleRow
    accumulations commute (plain DVE adds, any order).
  - DMA issue costs ~590ns ON THE ISSUING ENGINE'S QUEUE: the scalar
    (ACT) HWDGE queue carries only pre-attention traffic (weights, tail
    prefetch — its issue cost drains during phase Q) plus post-attention
    stores; xs/staging/attn_t/norm ride sync; the a2a-gated rhs8 loads
    ride gpsimd SWDGE (on a HWDGE queue the list scheduler hoists them
    and parks the queue on the collective semaphore).
  - the ACT exp table load (~1.3us) is prepaid with a dummy exp at t0.

The projection is split: out = f @ W_proj + attn2gcn @ W_proj + b.
  - attn2gcn (avn) rides the AllToAll as bf16 (x32; fp8 collectives
    measured pathologically slow), is cast to fp8e4 after the reshard,
    and the contraction runs as fp8 DoubleRow matmuls.
  - scales: f, W_proj staged x32 (bf16), avn x32 (fp8) -> psum carries
    1024*out; the host divides by 1024 after gathering (pure numpy).
"""

import numpy as np

B, N, C, H, D = 2, 2048, 1024, 16, 64
BN = B * N
SCALE = D ** -0.5
N_CORES = 8
KT = C // 128      # 8 contraction tiles
NCH = BN // 512    # 8 qkv free chunks
PS = 32.0          # fp8/bf16 staging scale for the projection operands

_cached = None


def _build():
    from contextlib import ExitStack

    import concourse.mybir as mybir
    import concourse.tile as tile
    from concourse import bacc
    from concourse.masks import make_identity

    F32 = mybir.dt.float32
    BF16 = mybir.dt.bfloat16
    F8 = mybir.dt.float8e4
    EXP = mybir.ActivationFunctionType.Exp
    DR = mybir.MatmulPerfMode.DoubleRow
    ADD = mybir.AluOpType.add
    MULT = mybir.AluOpType.mult

    nc = bacc.Bacc("TRN2", target_bir_lowering=False, debug=False,
                   num_devices=N_CORES)

    xT_ext = nc.dram_tensor("xT", [C, BN], BF16, kind="ExternalInput").ap()
    wkqv_ext = nc.dram_tensor("wkqv", [C, 384], BF16, kind="ExternalInput").ap()
    fT_ext = nc.dram_tensor("fT", [C, 512], BF16, kind="ExternalInput").ap()
    wproj_ext = nc.dram_tensor("wproj", [C, C], BF16, kind="ExternalInput").ap()
    wp8_ext = [nc.dram_tensor(f"wp8_{h}", [128, 4 * C], mybir.dt.float8e4,
                              kind="ExternalInput").ap() for h in range(2)]
    bprojT_ext = nc.dram_tensor("bprojT", [128, 8], F32, kind="ExternalInput").ap()
    attn_t_ext = nc.dram_tensor("attn_t", [128, BN], BF16, kind="ExternalOutput").ap()
    out_t_ext = nc.dram_tensor("out_t", [C, 512], BF16, kind="ExternalOutput").ap()

    groups = [list(range(N_CORES))]

    with tile.TileContext(nc) as tc:
        with ExitStack() as octx:
            pp = octx.enter_context(tc.tile_pool(name="persist", bufs=1))
            kqp = octx.enter_context(tc.tile_pool(name="kq", bufs=1))
            vap = octx.enter_context(tc.tile_pool(name="vaug", bufs=1))
            vtp = octx.enter_context(tc.tile_pool(name="vt", bufs=1))
            wqp = octx.enter_context(tc.tile_pool(name="wq", bufs=1))
            xsp = octx.enter_context(tc.tile_pool(name="xs", bufs=4))
            oaccp = octx.enter_context(tc.tile_pool(name="oacc", bufs=1))
            dram = octx.enter_context(
                tc.tile_pool(name="dram", bufs=1, space="DRAM"))
            # startup DMA: batched 3D descriptors — DMA issue costs ~600ns
            # of queue time each, and per-k-tile loads (~60 issues) were
            # measured to serialize the whole 43us startup.  One descriptor
            # per tensor/chunk: [p, k, cols] gathers all 8 k-tiles.
            xT_v = xT_ext.rearrange("(k p) t -> p k t", p=128)
            wq_sb = wqp.tile([128, KT, 384], BF16, name="wq")
            nc.scalar.dma_start(
                wq_sb[:], wkqv_ext.rearrange("(k p) m -> p k m", p=128))
            xs0_t = xsp.tile([128, KT, 512], BF16, name="xs", tag="xs")
            nc.sync.dma_start(xs0_t[:], xT_v[:, :, 0:512])
            xs1_t = xsp.tile([128, KT, 512], BF16, name="xs", tag="xs")
            nc.sync.dma_start(xs1_t[:], xT_v[:, :, 512:1024])

            # small persistent scratch + warmup
            ident = pp.tile([128, 128], BF16)
            make_identity(nc, ident[:])
            # 32, not 1: the normalization broadcast then yields avn*32
            # directly (the x32 the a2a/proj stage wants); the host divides
            # the attn output by 32 after gathering.
            ones64b = pp.tile([1, 64], BF16)
            nc.vector.memset(ones64b[:], PS)
            # lhs for the last chunk's DMA-free norm: broadcasting the raw
            # denominator row with weight PS/1024 gives bc_d = d/32, whose
            # fast-reciprocal is directly the 32/d the avn products need
            ones64r = pp.tile([1, 64], BF16)
            nc.vector.memset(ones64r[:], PS / 1024.0)

            # prepay the ACT exp-table load (~1.3us) before the first real
            # exp; the scalar queue is idle during phase Q anyway
            dummy = pp.tile([8, 8], BF16, name="dummy")
            nc.vector.memset(dummy[:], 0.0)
            dummy2 = pp.tile([8, 8], BF16, name="dummy2")
            nc.scalar.activation(dummy2[:], dummy[:], EXP)

            # bias for the projection accumulators (tiny, scalar queue)
            bias_sb = pp.tile([128, 8], F32)
            nc.scalar.dma_start(bias_sb[:], bprojT_ext[:])

            kT = kqp.tile([128, BN], BF16, name="kT")
            qT = kqp.tile([128, BN], BF16, name="qT")
            vT = vtp.tile([128, BN], BF16, name="vT")
            mtiles = [kT, qT, vT]
            v_aug = [[vap.tile([128, 65], BF16, name=f"va{h}_{j}")
                      for j in range(32)] for h in range(2)]

            # out_acc[m] starts at b_proj*PS*PS; f@W and the h0 DoubleRow
            # passes then += into it in any order (DVE adds commute)
            out_acc = [oaccp.tile([128, 512], F32, name=f"oacc{m}")
                       for m in range(8)]
            for m in range(8):
                nc.vector.memset(out_acc[m][:], 0.0)
                nc.vector.tensor_scalar_add(
                    out_acc[m][:], out_acc[m][:], bias_sb[:, m:m + 1])

            # avn is quantized to fp8 before the reshard; the collective
            # itself runs on a bf16 VIEW of those bytes ([512, 256] bf16 ==
            # [512, 512] fp8) — fp8-dtype collectives measured ~20x slower,
            # and this also removes any post-a2a convert from the tail.
            a2a_in = [dram.tile([512, 256], BF16, name=f"a2ain{hh}")
                      for hh in range(2)]
            a2a_out = [dram.tile([512, 256], BF16, name=f"a2aout{hh}")
                       for hh in range(2)]

            # tiny warmup collective: the first AllToAll of a NEFF pays a
            # large CC-stream start delay; paying it here (overlapped with
            # phase Q / the runtime barrier) takes it off the mid-kernel
            # critical path
            warm_in = dram.tile([8, 256], BF16, name="warm_in")
            warm_out = dram.tile([8, 256], BF16, name="warm_out")
            warm_sb = pp.tile([8, 256], BF16, name="warm_sb")
            nc.vector.memset(warm_sb[:], 0.0)
            nc.sync.dma_start(warm_in[:], warm_sb[:])
            nc.gpsimd.collective_compute(
                "AllToAll", mybir.AluOpType.bypass,
                replica_groups=groups,
                ins=[warm_in.opt()], outs=[warm_out.opt()])

            def qkv_xs(nch):
                xs = xsp.tile([128, KT, 512], BF16, name="xs", tag="xs")
                nc.sync.dma_start(
                    xs[:], xT_v[:, :, nch * 512:(nch + 1) * 512])
                return xs

            xs_cache = {0: xs0_t, 1: xs1_t}

            def transp_j(j, tpool, ttag):
                """v transpose for key-tile j -> v_aug[h][j] (+ones col)."""
                tps = tpool.tile([128, 128], BF16, name="tps", tag=ttag)
                nc.tensor.transpose(
                    tps[:], vT[:, j * 128:(j + 1) * 128], ident[:])
                for h in range(2):
                    nc.vector.tensor_copy(
                        v_aug[h][j][:, 0:64], tps[:, h * 64:(h + 1) * 64])
                    nc.vector.memset(v_aug[h][j][:, 64:65], 1.0)

            # ------------- phase Q: full qkv for chunks 0-1 -------------
            with ExitStack() as qctx:
                qps = qctx.enter_context(
                    tc.tile_pool(name="qkv_ps", bufs=1, space="PSUM"))
                trp = qctx.enter_context(
                    tc.tile_pool(name="tr_ps", bufs=2, space="PSUM"))
                # HAM warm-up: ~64 tiny identity matmuls keep the PE busy
                # through the input DMA ramp so the clock governor reaches
                # full rate before the real phase-Q matmuls start (cold
                # phase Q measured ~2x slower: ~600ns per 512-col MM)
                for _ in range(48):
                    bp = trp.tile([128, 128], F32, name="burn", tag="burn")
                    nc.tensor.matmul(bp[:], ident[:], ident[:],
                                     start=True, stop=True)
                for nch in range(2):
                    if nch not in xs_cache:
                        xs_cache[nch] = qkv_xs(nch)
                    xs_t = xs_cache[nch]
                    psums = [qps.tile([128, 512], F32, name=f"qps{m}",
                                      tag=f"qps{m}") for m in range(3)]
                    for k in range(KT):
                        for m in range(3):
                            nc.tensor.matmul(
                                psums[m][:],
                                wq_sb[:, k, m * 128:(m + 1) * 128],
                                xs_t[:, k, :], start=(k == 0),
                                stop=(k == KT - 1))
                    for m in range(3):
                        nc.vector.tensor_copy(
                            mtiles[m][:, nch * 512:(nch + 1) * 512],
                            psums[m][:])
                    for j in range(4 * nch, 4 * nch + 4):
                        transp_j(j, trp, "tps")

            # ---------------- phase A: attention + fillers ----------------
            with ExitStack() as actx:
                expp = actx.enter_context(tc.tile_pool(name="exp", bufs=4))
                avup = actx.enter_context(tc.tile_pool(name="avu", bufs=3))
                normp = actx.enter_context(tc.tile_pool(name="norm", bufs=2))
                avnp = actx.enter_context(tc.tile_pool(name="avn", bufs=2))
                wpp = actx.enter_context(tc.tile_pool(name="wp", bufs=1))
                wp8p = actx.enter_context(tc.tile_pool(name="wp8", bufs=1))
                rhs8p = actx.enter_context(tc.tile_pool(name="rhs8", bufs=1))
                sps = actx.enter_context(
                    tc.tile_pool(name="scores_ps", bufs=2, space="PSUM"))
                avps = actx.enter_context(
                    tc.tile_pool(name="av_ps", bufs=1, space="PSUM"))
                bcps = actx.enter_context(
                    tc.tile_pool(name="bc_ps", bufs=1, space="PSUM"))
                pjps = actx.enter_context(
                    tc.tile_pool(name="pj_ps", bufs=1, space="PSUM"))

                # tail prefetch: single 3D descriptors on the scalar HWDGE
                # queue — issue cost drains during phase Q, sync stays
                # clear for the filler xs loads.
                fT_sb = pp.tile([128, KT, 512], BF16, name="fTsb")
                nc.scalar.dma_start(
                    fT_sb[:], fT_ext.rearrange("(k p) t -> p k t", p=128))
                wp_sb = wpp.tile([128, KT, C], BF16, name="wp")
                nc.scalar.dma_start(
                    wp_sb[:], wproj_ext.rearrange("(k p) m -> p k m", p=128))
                wp8_sb = []
                for hh in range(2):
                    w8 = wp8p.tile([128, 4, C], F8, name=f"wp8_{hh}")
                    nc.scalar.dma_start(w8[:], wp8_ext[hh][:])
                    wp8_sb.append(w8)

                pe_work = []   # 4-m-tile DoubleRow units for the h0 half
                rhs8_sb = {}

                # ---- fine-grained PE filler stream -----------------------
                # Yields callables; each emits ONE PE instruction (plus a
                # trailing copy on gpsimd when a pass completes, so the DVE
                # queue never gates the kT/qT/vT availability).  Order is
                # deadline-driven for the b-outer chunk sequence:
                #   ci0 (h0,b0,c0): kT/v ch2 by kj8, kT/v ch3 by kj12,
                #                   qT ch2,3 by end of ci0 (ci1 reads them)
                #   ci4+ (b1): everything of ch4-7 — 3 chunks of slack
                def filler_gen():
                    # per chunk the LAST pass releases its xs tiles; for
                    # ch2/3 the k and v passes lead (scores/av of ci0 kj8+
                    # consume them) and q trails (first read at ci1)
                    order = [(2, 0), (2, 2), (3, 0), (3, 2), (2, 1), (3, 1),
                             (4, 0), (4, 1), (4, 2), (5, 0), (5, 1), (5, 2),
                             (6, 0), (6, 1), (6, 2), (7, 0), (7, 1), (7, 2)]
                    last_m = {2: 1, 3: 1, 4: 2, 5: 2, 6: 2, 7: 2}
                    for nch, m in order:
                        if nch not in xs_cache:
                            xs_cache[nch] = qkv_xs(nch)
                        xs_t = xs_cache[nch]
                        pjt = pjps.tile([128, 512], F32, name="qkvd",
                                        tag="pj")
                        for k in range(KT):
                            yield lambda k=k, m=m, pjt=pjt, xs_t=xs_t: \
                                nc.tensor.matmul(
                                    pjt[:],
                                    wq_sb[:, k, m * 128:(m + 1) * 128],
                                    xs_t[:, k, :], start=(k == 0),
                                    stop=(k == KT - 1))
                        def finish(nch=nch, m=m, pjt=pjt):
                            nc.vector.tensor_copy(
                                mtiles[m][:, nch * 512:(nch + 1) * 512],
                                pjt[:])
                            if m == last_m[nch]:
                                del xs_cache[nch]
                        yield finish
                        if m == 2:
                            for j in range(4 * nch, 4 * nch + 4):
                                yield lambda j=j: transp_j(j, bcps, "bc")

                fillers = filler_gen()
                fillers_done = [False]

                def fill(nmm):
                    for _ in range(nmm):
                        step = next(fillers, None)
                        if step is None:
                            fillers_done[0] = True
                            return
                        step()

                def load_rhs8(hh):
                    """Stack the 8 received [64,512] bf16 tiles of half hh
                    into two DoubleRow rhs tiles [128, 2, 512] (bf16 view,
                    bytes are already fp8).  These DMAs carry a wait on the
                    a2a output; they ride the gpsimd SWDGE queue, where the
                    only thing they can park is a later collective trigger
                    (which waits on the same a2a anyway).  On a HWDGE queue
                    the list scheduler hoists them ahead of mid-loop
                    staging/attn_t DMAs and parks the whole queue on the
                    collective semaphore (measured: a 15us ci7 stall)."""
                    tiles = []
                    for u in range(2):
                        r = rhs8p.tile([128, 2, 512], F8, name=f"r8_{hh}{u}")
                        for i in range(2):
                            s = 2 * u + i
                            if hh == 0:
                                eng = nc.gpsimd
                            else:
                                # final half: parking the HWDGE queues is
                                # free (everything behind these loads also
                                # waits on the a2a), and HWDGE issue is
                                # faster than SWDGE on the critical tail
                                eng = nc.sync if i == 0 else nc.scalar
                            eng.dma_start(
                                r[:, i, :].bitcast(BF16),
                                a2a_out[hh][s * 128:(s + 1) * 128, :])
                        tiles.append(r)
                    rhs8_sb[hh] = tiles

                def proj8_unit(hh, ms, final):
                    """DoubleRow avn@W for head-parity half hh, m-tiles ms."""
                    if hh not in rhs8_sb:
                        load_rhs8(hh)
                    r8 = rhs8_sb[hh]
                    w8 = wp8_sb[hh]
                    for m in ms:
                        pj = pjps.tile([128, 512], F32, name="pj8", tag="pj")
                        for u in range(2):
                            nc.tensor.matmul(
                                pj[:], w8[:, 2 * u:2 * u + 2,
                                          m * 128:(m + 1) * 128],
                                r8[u][:], start=(u == 0), stop=(u == 1),
                                perf_mode=DR)
                        if final:
                            ot = avnp.tile([128, 512], BF16, name="ot",
                                           tag="ot")
                            nc.vector.tensor_tensor(
                                ot[:], pj[:], out_acc[m][:], ADD)
                            eng = nc.sync if m % 2 == 0 else nc.scalar
                            eng.dma_start(
                                out_t_ext[m * 128:(m + 1) * 128, :], ot[:])
                        else:
                            nc.vector.tensor_tensor(
                                out_acc[m][:], pj[:], out_acc[m][:], ADD)

                def fw_tail():
                    """f @ W_proj: dependency-free bf16 matmuls, emitted in
                    the tail to cover the final AllToAll window (real work
                    instead of burn matmuls; keeps the HAM governor warm
                    for the DoubleRow contraction that follows)."""
                    for m in range(8):
                        pj = pjps.tile([128, 512], F32, name="pjf", tag="pj")
                        for k in range(KT):
                            nc.tensor.matmul(
                                pj[:], wp_sb[:, k, m * 128:(m + 1) * 128],
                                fT_sb[:, k, :], start=(k == 0),
                                stop=(k == KT - 1))
                        nc.vector.tensor_tensor(
                            out_acc[m][:], pj[:], out_acc[m][:], ADD)

                def norm_pre(avu):
                    """1/denom chain — latency starts at chunk end.  The
                    reciprocal runs in a [128, 8] spread of the denominator
                    row (DVE cost is free-size only: 8 cycles, not 1024);
                    both DMAs use the same p-major element order.  (A
                    direct [1,1024] reciprocal_approx_fast produced garbage
                    on hardware — the custom-DVE ucode appears to assume a
                    multi-partition layout — while simulating correctly.)"""
                    dn = normp.tile([128, 8], F32, name="dn", tag="dn")
                    nc.sync.dma_start(dn[:], avu[64:65, :])
                    dninv = normp.tile([128, 8], F32, name="dninv",
                                       tag="dninv")
                    nc.vector.reciprocal_approx_fast(dninv[:], dn[:])
                    dnb = normp.tile([128, 8], BF16, name="dnb", tag="dnb")
                    # on DVE, not ACT: an ACT copy would make every next
                    # chunk's exps queue behind this chain (ACT is in-order)
                    nc.vector.tensor_copy(dnb[:], dninv[:])
                    dinvb = normp.tile([1, 1024], BF16, name="dinvb",
                                       tag="dinvb")
                    nc.sync.dma_start(dinvb[:], dnb[:])
                    return dinvb

                def norm_chunk(h, b, ch, avu, dinvb, last=False):
                    """avn32 = 32*avu[0:64]/avu[64]; attn_t out (x32, the
                    host divides); bf16 a2a staging (x32 by design).

                    last=True: the final chunk gates the final AllToAll, so
                    its chain must not touch the sync HWDGE queue — the list
                    scheduler's cost model runs ~40us optimistic by then and
                    parks the queue on any DMA it hoists (measured 45us).
                    Instead: broadcast the RAW denominator row via ones64r
                    (scale-folded), fast-reciprocal on the [64,512]
                    broadcast (multi-partition, same shape class as the
                    proven path), and stage via gpsimd SWDGE, where any
                    parking resolves at a2a(0)-done anyway."""
                    po = h * 64
                    cs = b * 2048 + ch * 1024
                    avn32 = avnp.tile([64, 1024], BF16, name="avn32",
                                      tag="avn32")
                    avn8 = avnp.tile([64, 1024], F8, name="avn8", tag="avn8")
                    if last:
                        drow = normp.tile([1, 1024], BF16, name="drow",
                                          tag="drow")
                        nc.vector.tensor_copy(drow[:], avu[64:65, :])
                    for s in range(2):
                        bc = bcps.tile([64, 512], F32, name="bc", tag="bc")
                        if last:
                            nc.tensor.matmul(bc[:], ones64r[:],
                                             drow[:, s * 512:(s + 1) * 512],
                                             start=True, stop=True)
                            rcp = normp.tile([64, 512], F32, name="rcp",
                                             tag="rcp")
                            nc.vector.reciprocal_approx_fast(rcp[:], bc[:])
                            scl = rcp
                        else:
                            nc.tensor.matmul(bc[:], ones64b[:],
                                             dinvb[:, s * 512:(s + 1) * 512],
                                             start=True, stop=True)
                            scl = bc
                        # fp8 staging product first: the collective waits
                        # on it, the bf16 attn copy can trail
                        nc.vector.tensor_tensor(
                            avn8[:, s * 512:(s + 1) * 512],
                            avu[0:64, s * 512:(s + 1) * 512],
                            scl[:], MULT)
                        nc.vector.tensor_tensor(
                            avn32[:, s * 512:(s + 1) * 512],
                            avu[0:64, s * 512:(s + 1) * 512],
                            scl[:], MULT)
                    st_eng = nc.gpsimd if last else nc.sync
                    for j in (cs // 512, cs // 512 + 1):
                        off = j * 512 - cs
                        st_eng.dma_start(
                            a2a_in[h][j * 64:(j + 1) * 64, :],
                            avn8[:, off:off + 512].bitcast(BF16))
                    st_eng.dma_start(
                        attn_t_ext[po:po + 64, cs:cs + 1024], avn32[:])

                def mm1_kj(h, b, cs, kj, scores_q):
                    po = h * 64
                    jt = b * 16 + kj
                    sc = sps.tile([128, 1024], F32, name="scores", tag="sc")
                    for s in range(2):
                        nc.tensor.matmul(
                            sc[:, s * 512:(s + 1) * 512],
                            kT[po:po + 64, jt * 128:(jt + 1) * 128],
                            qT[po:po + 64, cs + s * 512:cs + (s + 1) * 512],
                            start=True, stop=True)
                    scores_q[kj] = sc

                def fire_a2a(h):
                    nc.gpsimd.collective_compute(
                        "AllToAll", mybir.AluOpType.bypass,
                        replica_groups=groups,
                        ins=[a2a_in[h].opt()], outs=[a2a_out[h].opt()])
                    if h == 0:
                        pe_work.extend([(0, range(0, 4)), (0, range(4, 8))])

                def do_chunk(ci, h, b, ch, pending):
                    cs = b * 2048 + ch * 1024
                    av = avps.tile([128, 1024], F32, name="av", tag="av")
                    scores_q = {}
                    mm1_kj(h, b, cs, 0, scores_q)
                    for kj in range(16):
                        if kj + 1 < 16:
                            mm1_kj(h, b, cs, kj + 1, scores_q)
                        if kj == 4 and pending is not None:
                            pci = pending[0]
                            norm_chunk(*pending[1:])
                            pending = None
                            if pci == 5:
                                # h0's 4th chunk is normed: its a2a can go
                                fire_a2a(0)
                        if not fillers_done[0]:
                            fill(4 if ci == 0 else (3 if ci < 4 else 1))
                        sc = scores_q.pop(kj)
                        ex = expp.tile([128, 1024], BF16, name="ex", tag="ex")
                        nc.scalar.activation(ex[:], sc[:], EXP, scale=SCALE)
                        if ci == 7 and kj == 15:
                            last_ex[0] = ex
                        jt = b * 16 + kj
                        for s in range(2):
                            nc.tensor.matmul(
                                av[0:65, s * 512:(s + 1) * 512],
                                v_aug[h][jt][:],
                                ex[:, s * 512:(s + 1) * 512],
                                start=(kj == 0), stop=(kj == 15))
                    avu = avup.tile([65, 1024], F32, name="avu", tag="avu")
                    nc.vector.tensor_copy(avu[:], av[0:65, :])
                    # the last chunk normalizes via the DMA-free path
                    dinvb = None if ci == 7 else norm_pre(avu)
                    return (ci, h, b, ch, avu, dinvb)

                # batch-outer chunk order; h0 finishes at ci=5
                seq = [(0, 0, 0), (0, 0, 1), (1, 0, 0), (1, 0, 1),
                       (0, 1, 0), (0, 1, 1), (1, 1, 0), (1, 1, 1)]
                pending = None
                last_ex = [None]
                for ci, (h, b, ch) in enumerate(seq):
                    pending = do_chunk(ci, h, b, ch, pending)
                # tail: drain leftover fillers, last chunk's norm, fire the
                # final collective, cover its transfer with f@W + the h0
                # DoubleRow units, then the h1 contraction + output stores.
                # ORDER MATTERS: fire_a2a(1) must be emitted BEFORE anything
                # gated on a2a(0) (rhs8 loads / DoubleRow-h0) — the list
                # scheduler otherwise queues the a2a(1) trigger behind them
                # on gpsimd and delays it until a2a(0) completes (~mid-ci7
                # under the batch-outer chunk order; measured +13us).
                fill(1 << 30)
                norm_chunk(*pending[1:], last=True)
                fire_a2a(1)
                # anchor: a throwaway matmul reading the LAST exp's output
                # into the shared pj psum ring.  The ring's WAR chain then
                # sequences every f@W / DoubleRow pass AFTER the attention
                # finishes — a REAL dependency the list scheduler must
                # honor.  Without it the scheduler front-runs f@W into the
                # ci4-6 exp slack (inflating those chunks) and the
                # a2a(0)-gated DoubleRow into ci7's PE queue (measured 9us
                # exp stall + cold-clock tail); the tail cover then runs
                # exactly where designed: over the final AllToAll window.
                pja = pjps.tile([128, 512], F32, name="pja", tag="pj")
                nc.tensor.matmul(pja[0:64, :], ones64b[:],
                                 last_ex[0][0:1, 0:512],
                                 start=True, stop=True)
                fw_tail()
                while pe_work:
                    proj8_unit(*pe_work.pop(0), final=False)
                proj8_unit(1, range(8), final=True)

    nc.compile()
    return nc


def kernel(x, f, W_qkv, W_proj, b_proj):
    import ml_dtypes
    from concourse.bass_utils import run_bass_kernel_spmd

    global _cached
    if _cached is None:
        _cached = _build()
    nc = _cached

    BF = ml_dtypes.bfloat16
    F8 = ml_dtypes.float8_e4m3
    x = np.ascontiguousarray(np.asarray(x, dtype=np.float32))
    f = np.ascontiguousarray(np.asarray(f, dtype=np.float32))
    W_qkv = np.asarray(W_qkv, dtype=np.float32)
    W_proj = np.asarray(W_proj, dtype=np.float32)
    b_proj = np.asarray(b_proj, dtype=np.float32)

    Wq, Wk, Wv = W_qkv[:, 0:C], W_qkv[:, C:2 * C], W_qkv[:, 2 * C:3 * C]
    xT = np.ascontiguousarray(x.reshape(BN, C).T.astype(BF))
    fT = (f.reshape(BN, C).T * PS).astype(BF)
    wproj_b = np.ascontiguousarray((W_proj * PS).astype(BF))
    bprojT = np.ascontiguousarray((b_proj * PS * PS).reshape(8, 128).T)

    # DoubleRow fp8 W_proj halves: wp8_h[p, s, m] = (W_proj*PS)[row, m] with
    # row = head*64 + p%64, head = 4s + h + 2*(p//64)
    Wp32 = W_proj * PS
    p = np.arange(128)
    wp8 = []
    for h in range(2):
        rows = np.empty((128, 4), np.int64)
        for s in range(4):
            head = 4 * s + h + 2 * (p // 64)
            rows[:, s] = head * 64 + (p % 64)
        wp8.append(np.ascontiguousarray(
            Wp32[rows, :].astype(F8).reshape(128, 4 * C)))

    in_maps = []
    for c in range(N_CORES):
        cols = slice(c * 128, (c + 1) * 128)     # heads 2c, 2c+1
        wkqv = np.ascontiguousarray(np.concatenate(
            [Wk[:, cols], Wq[:, cols], Wv[:, cols]], axis=1).astype(BF))
        in_maps.append({
            "xT": xT,
            "wkqv": wkqv,
            "fT": np.ascontiguousarray(fT[:, c * 512:(c + 1) * 512]),
            "wproj": wproj_b,
            "wp8_0": wp8[0],
            "wp8_1": wp8[1],
            "bprojT": bprojT,
        })

    res = run_bass_kernel_spmd(nc, in_maps, core_ids=list(range(N_CORES)))

    attn = np.empty((BN, C), dtype=np.float32)
    out = np.empty((BN, C), dtype=np.float32)
    for c in range(N_CORES):
        r = res.results[c]
        attn[:, c * 128:(c + 1) * 128] = r["attn_t"].T.astype(np.float32) / PS
        out[c * 512:(c + 1) * 512, :] = \
            r["out_t"].T.astype(np.float32) / (PS * PS)
    return out.reshape(B, N, C), attn.reshape(B, N, C)


# revision 38
# speedup vs baseline: 1.1687x; 1.0362x over previous
"""Distributed MHA kernel for Trainium2 (8 NeuronCores).

Problem: x,f:(2,2048,1024), W_qkv:(1024,3072), W_proj:(1024,1024), H=16 heads.
reference returns (out, attn2gcn) with
  attn2gcn = softmax(q k^T / sqrt(64)) v   (per head, concat over heads)
  out      = (attn2gcn + f) @ W_proj + b_proj

Sharding: tensor-parallel over heads — core c owns heads 2c, 2c+1 for both
batches (column block c*128 of the hidden dim).  Attention arithmetic is
bf16 matmuls with fp32 PSUM (the attn output's max-abs rel-err budget is
too tight for fp8 anywhere on that path: ex/v at e4m3 alone would cost
~1.7e-2 of the 2e-2 budget); softmax stays fp32 on ACT.

Timeline model (v2): the attention phase is a balanced PE/ACT race — ACT
exp costs (1024+352)/1.2GHz = 1113ns per kj and the core PE work (2 scores
+ 2 av matmuls, F=512) is ~1050ns/kj, so every extra PE cycle in the
window extends the kernel ~1:1.  Design consequences:
  - chunk order is batch-outer (b0: h0c0,h0c1,h1c0,h1c1; then b1) so the
    deferred qkv for batch 1 has 4 chunks of slack; only chunks 0-1 run
    eagerly in phase Q.  Deferred qkv is injected ONE matmul at a time
    (not 8-matmul bursts) between the ACT-feeding mm1 and the av matmuls.
  - h0's AllToAll fires after its last chunk (ci=5, during ci=6); h1's
    fires post-loop, its transfer covered by the f@W matmuls (real work
    that replaces the old burn matmuls and keeps the HAM clock warm).
  - out_acc[m] is bias-initialized at t0 so the f@W and h0-DoubleRow
    accumulations commute (plain DVE adds, any order).
  - DMA issue costs ~590ns ON THE ISSUING ENGINE'S QUEUE: the scalar
    (ACT) HWDGE queue carries only pre-attention traffic (weights, tail
    prefetch — its issue cost drains during phase Q) plus post-attention
    stores; xs/staging/attn_t/norm ride sync; the a2a-gated rhs8 loads
    ride gpsimd SWDGE (on a HWDGE queue the list scheduler hoists them
    and parks the queue on the collective semaphore).
  - the ACT exp table load (~1.3us) is prepaid with a dummy exp at t0.

The projection is split: out = f @ W_proj + attn2gcn @ W_proj + b.
  - attn2gcn (avn) rides the AllToAll as bf16 (x32; fp8 collectives
    measured pathologically slow), is cast to fp8e4 after the reshard,
    and the contraction runs as fp8 DoubleRow matmuls.
  - scales: f, W_proj staged x32 (bf16), avn x32 (fp8) -> psum carries
    1024*out; the host divides by 1024 after gathering (pure numpy).
"""

import numpy as np

B, N, C, H, D = 2, 2048, 1024, 16, 64
BN = B * N
SCALE = D ** -0.5
N_CORES = 8
KT = C // 128      # 8 contraction tiles
NCH = BN // 512    # 8 qkv free chunks
PS = 32.0          # fp8/bf16 staging scale for the projection operands

_cached = None


def _build():
    from contextlib import ExitStack

    import concourse.mybir as mybir
    import concourse.tile as tile
    from concourse import bacc
    from concourse.masks import make_identity

    F32 = mybir.dt.float32
    BF16 = mybir.dt.bfloat16
    F8 = mybir.dt.float8e4
    EXP = mybir.ActivationFunctionType.Exp
    DR = mybir.MatmulPerfMode.DoubleRow
    ADD = mybir.AluOpType.add
    MULT = mybir.AluOpType.mult

    nc = bacc.Bacc("TRN2", target_bir_lowering=False, debug=False,
                   num_devices=N_CORES)

    xT_ext = nc.dram_tensor("xT", [C, BN], BF16, kind="ExternalInput").ap()
    wkqv_ext = nc.dram_tensor("wkqv", [C, 384], BF16, kind="ExternalInput").ap()
    fT_ext = nc.dram_tensor("fT", [C, 512], BF16, kind="ExternalInput").ap()
    wproj_ext = nc.dram_tensor("wproj", [C, C], BF16, kind="ExternalInput").ap()
    wp8_ext = [nc.dram_tensor(f"wp8_{h}", [128, 4 * C], mybir.dt.float8e4,
                              kind="ExternalInput").ap() for h in range(2)]
    bprojT_ext = nc.dram_tensor("bprojT", [128, 8], F32, kind="ExternalInput").ap()
    attn_t_ext = nc.dram_tensor("attn_t", [128, BN], BF16, kind="ExternalOutput").ap()
    out_t_ext = nc.dram_tensor("out_t", [C, 512], BF16, kind="ExternalOutput").ap()

    groups = [list(range(N_CORES))]

    with tile.TileContext(nc) as tc:
        with ExitStack() as octx:
            pp = octx.enter_context(tc.tile_pool(name="persist", bufs=1))
            kqp = octx.enter_context(tc.tile_pool(name="kq", bufs=1))
            vap = octx.enter_context(tc.tile_pool(name="vaug", bufs=1))
            vtp = octx.enter_context(tc.tile_pool(name="vt", bufs=1))
            wqp = octx.enter_context(tc.tile_pool(name="wq", bufs=1))
            xsp = octx.enter_context(tc.tile_pool(name="xs", bufs=4))
            oaccp = octx.enter_context(tc.tile_pool(name="oacc", bufs=1))
            dram = octx.enter_context(
                tc.tile_pool(name="dram", bufs=1, space="DRAM"))
            # startup DMA: batched 3D descriptors — DMA issue costs ~600ns
            # of queue time each, and per-k-tile loads (~60 issues) were
            # measured to serialize the whole 43us startup.  One descriptor
            # per tensor/chunk: [p, k, cols] gathers all 8 k-tiles.
            xT_v = xT_ext.rearrange("(k p) t -> p k t", p=128)
            wq_sb = wqp.tile([128, KT, 384], BF16, name="wq")
            nc.scalar.dma_start(
                wq_sb[:], wkqv_ext.rearrange("(k p) m -> p k m", p=128))
            xs0_t = xsp.tile([128, KT, 512], BF16, name="xs", tag="xs")
            nc.sync.dma_start(xs0_t[:], xT_v[:, :, 0:512])
            xs1_t = xsp.tile([128, KT, 512], BF16, name="xs", tag="xs")
            nc.sync.dma_start(xs1_t[:], xT_v[:, :, 512:1024])

            # small persistent scratch + warmup
            ident = pp.tile([128, 128], BF16)
            make_identity(nc, ident[:])
            # 32, not 1: the normalization broadcast then yields avn*32
            # directly (the x32 the a2a/proj stage wants); the host divides
            # the attn output by 32 after gathering.
            ones64b = pp.tile([1, 64], BF16)
            nc.vector.memset(ones64b[:], PS)
            # lhs for the last chunk's DMA-free norm: broadcasting the raw
            # denominator row with weight PS/1024 gives bc_d = d/32, whose
            # fast-reciprocal is directly the 32/d the avn products need
            ones64r = pp.tile([1, 64], BF16)
            nc.vector.memset(ones64r[:], PS / 1024.0)

            # prepay the ACT exp-table load (~1.3us) before the first real
            # exp; the scalar queue is idle during phase Q anyway
            dummy = pp.tile([8, 8], BF16, name="dummy")
            nc.vector.memset(dummy[:], 0.0)
            dummy2 = pp.tile([8, 8], BF16, name="dummy2")
            nc.scalar.activation(dummy2[:], dummy[:], EXP)

            # bias for the projection accumulators (tiny, scalar queue)
            bias_sb = pp.tile([128, 8], F32)
            nc.scalar.dma_start(bias_sb[:], bprojT_ext[:])

            kT = kqp.tile([128, BN], BF16, name="kT")
            qT = kqp.tile([128, BN], BF16, name="qT")
            vT = vtp.tile([128, BN], BF16, name="vT")
            mtiles = [kT, qT, vT]
            v_aug = [[vap.tile([128, 65], BF16, name=f"va{h}_{j}")
                      for j in range(32)] for h in range(2)]

            # out_acc[m] starts at b_proj*PS*PS; f@W and the h0 DoubleRow
            # passes then += into it in any order (DVE adds commute)
            out_acc = [oaccp.tile([128, 512], F32, name=f"oacc{m}")
                       for m in range(8)]
            for m in range(8):
                nc.vector.memset(out_acc[m][:], 0.0)
                nc.vector.tensor_scalar_add(
                    out_acc[m][:], out_acc[m][:], bias_sb[:, m:m + 1])

            # avn is quantized to fp8 before the reshard; the collective
            # itself runs on a bf16 VIEW of those bytes ([512, 256] bf16 ==
            # [512, 512] fp8) — fp8-dtype collectives measured ~20x slower,
            # and this also removes any post-a2a convert from the tail.
            a2a_in = [dram.tile([512, 256], BF16, name=f"a2ain{hh}")
                      for hh in range(2)]
            a2a_out = [dram.tile([512, 256], BF16, name=f"a2aout{hh}")
                       for hh in range(2)]

            # tiny warmup collective: the first AllToAll of a NEFF pays a
            # large CC-stream start delay; paying it here (overlapped with
            # phase Q / the runtime barrier) takes it off the mid-kernel
            # critical path
            warm_in = dram.tile([8, 256], BF16, name="warm_in")
            warm_out = dram.tile([8, 256], BF16, name="warm_out")
            warm_sb = pp.tile([8, 256], BF16, name="warm_sb")
            nc.vector.memset(warm_sb[:], 0.0)
            nc.sync.dma_start(warm_in[:], warm_sb[:])
            nc.gpsimd.collective_compute(
                "AllToAll", mybir.AluOpType.bypass,
                replica_groups=groups,
                ins=[warm_in.opt()], outs=[warm_out.opt()])

            def qkv_xs(nch):
                xs = xsp.tile([128, KT, 512], BF16, name="xs", tag="xs")
                nc.sync.dma_start(
                    xs[:], xT_v[:, :, nch * 512:(nch + 1) * 512])
                return xs

            xs_cache = {0: xs0_t, 1: xs1_t}

            def transp_j(j, tpool, ttag):
                """v transpose for key-tile j -> v_aug[h][j] (+ones col)."""
                tps = tpool.tile([128, 128], BF16, name="tps", tag=ttag)
                nc.tensor.transpose(
                    tps[:], vT[:, j * 128:(j + 1) * 128], ident[:])
                for h in range(2):
                    nc.vector.tensor_copy(
                        v_aug[h][j][:, 0:64], tps[:, h * 64:(h + 1) * 64])
                    nc.vector.memset(v_aug[h][j][:, 64:65], 1.0)

            # ------------- phase Q: full qkv for chunks 0-1 -------------
            with ExitStack() as qctx:
                qps = qctx.enter_context(
                    tc.tile_pool(name="qkv_ps", bufs=1, space="PSUM"))
                trp = qctx.enter_context(
                    tc.tile_pool(name="tr_ps", bufs=2, space="PSUM"))
                # HAM warm-up: ~64 tiny identity matmuls keep the PE busy
                # through the input DMA ramp so the clock governor reaches
                # full rate before the real phase-Q matmuls start (cold
                # phase Q measured ~2x slower: ~600ns per 512-col MM)
                for _ in range(48):
                    bp = trp.tile([128, 128], F32, name="burn", tag="burn")
                    nc.tensor.matmul(bp[:], ident[:], ident[:],
                                     start=True, stop=True)
                for nch in range(2):
                    if nch not in xs_cache:
                        xs_cache[nch] = qkv_xs(nch)
                    xs_t = xs_cache[nch]
                    psums = [qps.tile([128, 512], F32, name=f"qps{m}",
                                      tag=f"qps{m}") for m in range(3)]
                    for k in range(KT):
                        for m in range(3):
                            nc.tensor.matmul(
                                psums[m][:],
                                wq_sb[:, k, m * 128:(m + 1) * 128],
                                xs_t[:, k, :], start=(k == 0),
                                stop=(k == KT - 1))
                    for m in range(3):
                        nc.vector.tensor_copy(
                            mtiles[m][:, nch * 512:(nch + 1) * 512],
                            psums[m][:])
                    for j in range(4 * nch, 4 * nch + 4):
                        transp_j(j, trp, "tps")

            # ---------------- phase A: attention + fillers ----------------
            with ExitStack() as actx:
                expp = actx.enter_context(tc.tile_pool(name="exp", bufs=4))
                avup = actx.enter_context(tc.tile_pool(name="avu", bufs=3))
                normp = actx.enter_context(tc.tile_pool(name="norm", bufs=2))
                avnp = actx.enter_context(tc.tile_pool(name="avn", bufs=2))
                wpp = actx.enter_context(tc.tile_pool(name="wp", bufs=1))
                wp8p = actx.enter_context(tc.tile_pool(name="wp8", bufs=1))
                rhs8p = actx.enter_context(tc.tile_pool(name="rhs8", bufs=1))
                sps = actx.enter_context(
                    tc.tile_pool(name="scores_ps", bufs=2, space="PSUM"))
                avps = actx.enter_context(
                    tc.tile_pool(name="av_ps", bufs=1, space="PSUM"))
                bcps = actx.enter_context(
                    tc.tile_pool(name="bc_ps", bufs=1, space="PSUM"))
                pjps = actx.enter_context(
                    tc.tile_pool(name="pj_ps", bufs=1, space="PSUM"))

                # tail prefetch: single 3D descriptors on the scalar HWDGE
                # queue — issue cost drains during phase Q, sync stays
                # clear for the filler xs loads.
                fT_sb = pp.tile([128, KT, 512], BF16, name="fTsb")
                nc.scalar.dma_start(
                    fT_sb[:], fT_ext.rearrange("(k p) t -> p k t", p=128))
                wp_sb = wpp.tile([128, KT, C], BF16, name="wp")
                nc.scalar.dma_start(
                    wp_sb[:], wproj_ext.rearrange("(k p) m -> p k m", p=128))
                wp8_sb = []
                for hh in range(2):
                    w8 = wp8p.tile([128, 4, C], F8, name=f"wp8_{hh}")
                    nc.scalar.dma_start(w8[:], wp8_ext[hh][:])
                    wp8_sb.append(w8)

                pe_work = []   # 4-m-tile DoubleRow units for the h0 half
                rhs8_sb = {}

                # ---- fine-grained PE filler stream -----------------------
                # Yields callables; each emits ONE PE instruction (plus a
                # trailing copy on gpsimd when a pass completes, so the DVE
                # queue never gates the kT/qT/vT availability).  Order is
                # deadline-driven for the b-outer chunk sequence:
                #   ci0 (h0,b0,c0): kT/v ch2 by kj8, kT/v ch3 by kj12,
                #                   qT ch2,3 by end of ci0 (ci1 reads them)
                #   ci4+ (b1): everything of ch4-7 — 3 chunks of slack
                def filler_gen():
                    # per chunk the LAST pass releases its xs tiles; for
                    # ch2/3 the k and v passes lead (scores/av of ci0 kj8+
                    # consume them) and q trails (first read at ci1)
                    order = [(2, 0), (2, 2), (3, 0), (3, 2), (2, 1), (3, 1),
                             (4, 0), (4, 1), (4, 2), (5, 0), (5, 1), (5, 2),
                             (6, 0), (6, 1), (6, 2), (7, 0), (7, 1), (7, 2)]
                    last_m = {2: 1, 3: 1, 4: 2, 5: 2, 6: 2, 7: 2}
                    for nch, m in order:
                        if nch not in xs_cache:
                            xs_cache[nch] = qkv_xs(nch)
                        xs_t = xs_cache[nch]
                        pjt = pjps.tile([128, 512], F32, name="qkvd",
                                        tag="pj")
                        for k in range(KT):
                            yield lambda k=k, m=m, pjt=pjt, xs_t=xs_t: \
                                nc.tensor.matmul(
                                    pjt[:],
                                    wq_sb[:, k, m * 128:(m + 1) * 128],
                                    xs_t[:, k, :], start=(k == 0),
                                    stop=(k == KT - 1))
                        def finish(nch=nch, m=m, pjt=pjt):
                            nc.vector.tensor_copy(
                                mtiles[m][:, nch * 512:(nch + 1) * 512],
                                pjt[:])
                            if m == last_m[nch]:
                                del xs_cache[nch]
                        yield finish
                        if m == 2:
                            for j in range(4 * nch, 4 * nch + 4):
                                yield lambda j=j: transp_j(j, bcps, "bc")

                fillers = filler_gen()
                fillers_done = [False]

                def fill(nmm):
                    for _ in range(nmm):
                        step = next(fillers, None)
                        if step is None:
                            fillers_done[0] = True
                            return
                        step()

                def load_rhs8(hh):
                    """Stack the 8 received [64,512] bf16 tiles of half hh
                    into two DoubleRow rhs tiles [128, 2, 512] (bf16 view,
                    bytes are already fp8).  These DMAs carry a wait on the
                    a2a output; they ride the gpsimd SWDGE queue, where the
                    only thing they can park is a later collective trigger
                    (which waits on the same a2a anyway).  On a HWDGE queue
                    the list scheduler hoists them ahead of mid-loop
                    staging/attn_t DMAs and parks the whole queue on the
                    collective semaphore (measured: a 15us ci7 stall)."""
                    tiles = []
                    for u in range(2):
                        r = rhs8p.tile([128, 2, 512], F8, name=f"r8_{hh}{u}")
                        for i in range(2):
                            s = 2 * u + i
                            if hh == 0:
                                eng = nc.gpsimd
                            else:
                                # final half: parking the HWDGE queues is
                                # free (everything behind these loads also
                                # waits on the a2a), and HWDGE issue is
                                # faster than SWDGE on the critical tail
                                eng = nc.sync if i == 0 else nc.scalar
                            eng.dma_start(
                                r[:, i, :].bitcast(BF16),
                                a2a_out[hh][s * 128:(s + 1) * 128, :])
                        tiles.append(r)
                    rhs8_sb[hh] = tiles

                # tail projection psum ring: alternate between the pj bank
                # and the bc bank (idle once attention ends) so unit m+1's
                # matmuls overlap unit m's DVE add — a single-bank ring
                # serializes the whole tail at ~1.8us per m-tile (measured).
                # Both rings still start late: pjps behind the exp anchor,
                # bcps behind the last chunk's norm broadcast.
                pj_ctr = [0]

                def pj_tile():
                    pj_ctr[0] += 1
                    if pj_ctr[0] % 2:
                        return pjps.tile([128, 512], F32, name="pj", tag="pj")
                    return bcps.tile([128, 512], F32, name="pjb", tag="bc")

                def proj8_unit(hh, ms, final):
                    """DoubleRow avn@W for head-parity half hh, m-tiles ms."""
                    if hh not in rhs8_sb:
                        load_rhs8(hh)
                    r8 = rhs8_sb[hh]
                    w8 = wp8_sb[hh]
                    for m in ms:
                        pj = pj_tile()
                        for u in range(2):
                            nc.tensor.matmul(
                                pj[:], w8[:, 2 * u:2 * u + 2,
                                          m * 128:(m + 1) * 128],
                                r8[u][:], start=(u == 0), stop=(u == 1),
                                perf_mode=DR)
                        if final:
                            ot = avnp.tile([128, 512], BF16, name="ot",
                                           tag="ot")
                            nc.vector.tensor_tensor(
                                ot[:], pj[:], out_acc[m][:], ADD)
                            eng = nc.sync if m % 2 == 0 else nc.scalar
                            eng.dma_start(
                                out_t_ext[m * 128:(m + 1) * 128, :], ot[:])
                        else:
                            nc.vector.tensor_tensor(
                                out_acc[m][:], pj[:], out_acc[m][:], ADD)

                def fw_tail():
                    """f @ W_proj: dependency-free bf16 matmuls, emitted in
                    the tail to cover the final AllToAll window (real work
                    instead of burn matmuls; keeps the HAM governor warm
                    for the DoubleRow contraction that follows)."""
                    for m in range(8):
                        pj = pj_tile()
                        for k in range(KT):
                            nc.tensor.matmul(
                                pj[:], wp_sb[:, k, m * 128:(m + 1) * 128],
                                fT_sb[:, k, :], start=(k == 0),
                                stop=(k == KT - 1))
                        nc.vector.tensor_tensor(
                            out_acc[m][:], pj[:], out_acc[m][:], ADD)

                def norm_pre(avu):
                    """1/denom chain — latency starts at chunk end.  The
                    reciprocal runs in a [128, 8] spread of the denominator
                    row (DVE cost is free-size only: 8 cycles, not 1024);
                    both DMAs use the same p-major element order.  (A
                    direct [1,1024] reciprocal_approx_fast produced garbage
                    on hardware — the custom-DVE ucode appears to assume a
                    multi-partition layout — while simulating correctly.)"""
                    dn = normp.tile([128, 8], F32, name="dn", tag="dn")
                    nc.sync.dma_start(dn[:], avu[64:65, :])
                    dninv = normp.tile([128, 8], F32, name="dninv",
                                       tag="dninv")
                    nc.vector.reciprocal_approx_fast(dninv[:], dn[:])
                    dnb = normp.tile([128, 8], BF16, name="dnb", tag="dnb")
                    # on DVE, not ACT: an ACT copy would make every next
                    # chunk's exps queue behind this chain (ACT is in-order)
                    nc.vector.tensor_copy(dnb[:], dninv[:])
                    dinvb = normp.tile([1, 1024], BF16, name="dinvb",
                                       tag="dinvb")
                    nc.sync.dma_start(dinvb[:], dnb[:])
                    return dinvb

                def norm_chunk(h, b, ch, avu, dinvb, last=False):
                    """avn32 = 32*avu[0:64]/avu[64]; attn_t out (x32, the
                    host divides); bf16 a2a staging (x32 by design).

                    last=True: the final chunk gates the final AllToAll, so
                    its chain must not touch the sync HWDGE queue — the list
                    scheduler's cost model runs ~40us optimistic by then and
                    parks the queue on any DMA it hoists (measured 45us).
                    Instead: broadcast the RAW denominator row via ones64r
                    (scale-folded), fast-reciprocal on the [64,512]
                    broadcast (multi-partition, same shape class as the
                    proven path), and stage via gpsimd SWDGE, where any
                    parking resolves at a2a(0)-done anyway."""
                    po = h * 64
                    cs = b * 2048 + ch * 1024
                    avn32 = avnp.tile([64, 1024], BF16, name="avn32",
                                      tag="avn32")
                    avn8 = avnp.tile([64, 1024], F8, name="avn8", tag="avn8")
                    if last:
                        # dinvb is the av PSUM tile here: read the raw
                        # denominator row straight from PSUM
                        drow = normp.tile([1, 1024], BF16, name="drow",
                                          tag="drow")
                        nc.vector.tensor_copy(drow[:], dinvb[64:65, :])
                    for s in range(2):
                        bc = bcps.tile([64, 512], F32, name="bc", tag="bc")
                        if last:
                            nc.tensor.matmul(bc[:], ones64r[:],
                                             drow[:, s * 512:(s + 1) * 512],
                                             start=True, stop=True)
                            rcp = normp.tile([64, 512], F32, name="rcp",
                                             tag="rcp")
                            nc.vector.reciprocal_approx_fast(rcp[:], bc[:])
                            scl = rcp
                        else:
                            nc.tensor.matmul(bc[:], ones64b[:],
                                             dinvb[:, s * 512:(s + 1) * 512],
                                             start=True, stop=True)
                            scl = bc
                        # fp8 staging product first: the collective waits
                        # on it, the bf16 attn copy can trail
                        nc.vector.tensor_tensor(
                            avn8[:, s * 512:(s + 1) * 512],
                            avu[0:64, s * 512:(s + 1) * 512],
                            scl[:], MULT)
                        nc.vector.tensor_tensor(
                            avn32[:, s * 512:(s + 1) * 512],
                            avu[0:64, s * 512:(s + 1) * 512],
                            scl[:], MULT)
                    st_eng = nc.gpsimd if last else nc.sync
                    for j in (cs // 512, cs // 512 + 1):
                        off = j * 512 - cs
                        st_eng.dma_start(
                            a2a_in[h][j * 64:(j + 1) * 64, :],
                            avn8[:, off:off + 512].bitcast(BF16))
                    st_eng.dma_start(
                        attn_t_ext[po:po + 64, cs:cs + 1024], avn32[:])

                def mm1_kj(h, b, cs, kj, scores_q):
                    po = h * 64
                    jt = b * 16 + kj
                    sc = sps.tile([128, 1024], F32, name="scores", tag="sc")
                    for s in range(2):
                        nc.tensor.matmul(
                            sc[:, s * 512:(s + 1) * 512],
                            kT[po:po + 64, jt * 128:(jt + 1) * 128],
                            qT[po:po + 64, cs + s * 512:cs + (s + 1) * 512],
                            start=True, stop=True)
                    scores_q[kj] = sc

                def fire_a2a(h):
                    nc.gpsimd.collective_compute(
                        "AllToAll", mybir.AluOpType.bypass,
                        replica_groups=groups,
                        ins=[a2a_in[h].opt()], outs=[a2a_out[h].opt()])
                    if h == 0:
                        pe_work.extend([(0, range(0, 4)), (0, range(4, 8))])

                def do_chunk(ci, h, b, ch, pending):
                    cs = b * 2048 + ch * 1024
                    av = avps.tile([128, 1024], F32, name="av", tag="av")
                    scores_q = {}
                    mm1_kj(h, b, cs, 0, scores_q)
                    for kj in range(16):
                        if kj + 1 < 16:
                            mm1_kj(h, b, cs, kj + 1, scores_q)
                        if kj == 4 and pending is not None:
                            pci = pending[0]
                            norm_chunk(*pending[1:])
                            pending = None
                            if pci == 5:
                                # h0's 4th chunk is normed: its a2a can go
                                fire_a2a(0)
                        if not fillers_done[0]:
                            fill(4 if ci == 0 else (3 if ci < 4 else 1))
                        sc = scores_q.pop(kj)
                        ex = expp.tile([128, 1024], BF16, name="ex", tag="ex")
                        nc.scalar.activation(ex[:], sc[:], EXP, scale=SCALE)
                        if ci == 7 and kj == 15:
                            last_ex[0] = ex
                        jt = b * 16 + kj
                        for s in range(2):
                            nc.tensor.matmul(
                                av[0:65, s * 512:(s + 1) * 512],
                                v_aug[h][jt][:],
                                ex[:, s * 512:(s + 1) * 512],
                                start=(kj == 0), stop=(kj == 15))
                    avu = avup.tile([65, 1024], F32, name="avu", tag="avu")
                    nc.vector.tensor_copy(avu[:], av[0:65, :])
                    # the last chunk normalizes via the DMA-free path; hand
                    # it the av PSUM tile so the denominator row is read
                    # directly (skips the avu-copy latency on the chain
                    # that gates the final AllToAll)
                    dinvb = av if ci == 7 else norm_pre(avu)
                    return (ci, h, b, ch, avu, dinvb)

                # batch-outer chunk order; h0 finishes at ci=5
                seq = [(0, 0, 0), (0, 0, 1), (1, 0, 0), (1, 0, 1),
                       (0, 1, 0), (0, 1, 1), (1, 1, 0), (1, 1, 1)]
                pending = None
                last_ex = [None]
                for ci, (h, b, ch) in enumerate(seq):
                    pending = do_chunk(ci, h, b, ch, pending)
                # tail: drain leftover fillers, last chunk's norm, fire the
                # final collective, cover its transfer with f@W + the h0
                # DoubleRow units, then the h1 contraction + output stores.
                # ORDER MATTERS: fire_a2a(1) must be emitted BEFORE anything
                # gated on a2a(0) (rhs8 loads / DoubleRow-h0) — the list
                # scheduler otherwise queues the a2a(1) trigger behind them
                # on gpsimd and delays it until a2a(0) completes (~mid-ci7
                # under the batch-outer chunk order; measured +13us).
                fill(1 << 30)
                norm_chunk(*pending[1:], last=True)
                fire_a2a(1)
                # anchor: a throwaway matmul reading the LAST exp's output
                # into the shared pj psum ring.  The ring's WAR chain then
                # sequences every f@W / DoubleRow pass AFTER the attention
                # finishes — a REAL dependency the list scheduler must
                # honor.  Without it the scheduler front-runs f@W into the
                # ci4-6 exp slack (inflating those chunks) and the
                # a2a(0)-gated DoubleRow into ci7's PE queue (measured 9us
                # exp stall + cold-clock tail); the tail cover then runs
                # exactly where designed: over the final AllToAll window.
                pja = pjps.tile([128, 512], F32, name="pja", tag="pj")
                nc.tensor.matmul(pja[0:64, :], ones64b[:],
                                 last_ex[0][0:1, 0:512],
                                 start=True, stop=True)
                fw_tail()
                while pe_work:
                    proj8_unit(*pe_work.pop(0), final=False)
                proj8_unit(1, range(8), final=True)

    nc.compile()
    return nc


def kernel(x, f, W_qkv, W_proj, b_proj):
    import ml_dtypes
    from concourse.bass_utils import run_bass_kernel_spmd

    global _cached
    if _cached is None:
        _cached = _build()
    nc = _cached

    BF = ml_dtypes.bfloat16
    F8 = ml_dtypes.float8_e4m3
    x = np.ascontiguousarray(np.asarray(x, dtype=np.float32))
    f = np.ascontiguousarray(np.asarray(f, dtype=np.float32))
    W_qkv = np.asarray(W_qkv, dtype=np.float32)
    W_proj = np.asarray(W_proj, dtype=np.float32)
    b_proj = np.asarray(b_proj, dtype=np.float32)

    Wq, Wk, Wv = W_qkv[:, 0:C], W_qkv[:, C:2 * C], W_qkv[:, 2 * C:3 * C]
    xT = np.ascontiguousarray(x.reshape(BN, C).T.astype(BF))
    fT = (f.reshape(BN, C).T * PS).astype(BF)
    wproj_b = np.ascontiguousarray((W_proj * PS).astype(BF))
    bprojT = np.ascontiguousarray((b_proj * PS * PS).reshape(8, 128).T)

    # DoubleRow fp8 W_proj halves: wp8_h[p, s, m] = (W_proj*PS)[row, m] with
    # row = head*64 + p%64, head = 4s + h + 2*(p//64)
    Wp32 = W_proj * PS
    p = np.arange(128)
    wp8 = []
    for h in range(2):
        rows = np.empty((128, 4), np.int64)
        for s in range(4):
            head = 4 * s + h + 2 * (p // 64)
            rows[:, s] = head * 64 + (p % 64)
        wp8.append(np.ascontiguousarray(
            Wp32[rows, :].astype(F8).reshape(128, 4 * C)))

    in_maps = []
    for c in range(N_CORES):
        cols = slice(c * 128, (c + 1) * 128)     # heads 2c, 2c+1
        wkqv = np.ascontiguousarray(np.concatenate(
            [Wk[:, cols], Wq[:, cols], Wv[:, cols]], axis=1).astype(BF))
        in_maps.append({
            "xT": xT,
            "wkqv": wkqv,
            "fT": np.ascontiguousarray(fT[:, c * 512:(c + 1) * 512]),
            "wproj": wproj_b,
            "wp8_0": wp8[0],
            "wp8_1": wp8[1],
            "bprojT": bprojT,
        })

    res = run_bass_kernel_spmd(nc, in_maps, core_ids=list(range(N_CORES)))

    attn = np.empty((BN, C), dtype=np.float32)
    out = np.empty((BN, C), dtype=np.float32)
    for c in range(N_CORES):
        r = res.results[c]
        attn[:, c * 128:(c + 1) * 128] = r["attn_t"].T.astype(np.float32) / PS
        out[c * 512:(c + 1) * 512, :] = \
            r["out_t"].T.astype(np.float32) / (PS * PS)
    return out.reshape(B, N, C), attn.reshape(B, N, C)
